# revision 1
# baseline (speedup 1.0000x reference)
"""Trainium2 Bass kernel for nn_CascadedVMambaBlock.

Sharding: 8 cores; core c = (b, nh) with b = c//4, nh = c%4.
Each core processes sample b with state-dim slice n in [4nh, 4nh+4)
for ALL 4 scan directions k. Per-head combine is a ReduceScatter over
the 4-core b-group (each rank keeps an L-shard of 576 pixels), the
out_norm/out_proj tail runs sharded, and an AllGather rebuilds the
full next-stage input.

Layouts: channels-first (channels on partitions, L = 2304 free).
Scan lanes are INTERLEAVED: partition p = 2*d + half covers channel
d = p//2, state n = 4*nh + 2*gl + half, so the per-(k,gl) B/C
row-expansions are 2-row periodic patterns built by log-doubling
SBUF->SBUF DMAs (fp16), letting the bB/hC multiplies run at DVE
2x_1p. xs is produced directly in duplicated-interleaved form by the
conv matmul (PE cost is free-dim cycles; extra output rows are free).
tensor_tensor_scan keeps fp32 internal state regardless of operand
dtype, so fp16 operands only round inputs/outputs.

LN1 + in_proj + depthwise conv3x3 run as THREE accumulated PE matmuls
per output chunk over dx-shifted zero-padded copies of
sr = s*rsqrt(var+eps) (fp16), with -mean*r correction rows folded in
as extra contraction rows; one fused Silu per output. All LN stats
matmuls use fp16 rhs (fp32r is 4x slower at unramped PE p-state).
"""
import numpy as np

HEAD, C_IN, C_H = 4, 128, 32
D, N, K, DT_RANK = 64, 16, 4, 2
B, H, W = 2, 48, 48
L = H * W            # 2304
CS = 512             # LN1 chunk
CHUNKS = [(i * CS, min(CS, L - i * CS)) for i in range((L + CS - 1) // CS)]
SCS = 1024           # scan chunk
SCHUNKS = [(i * SCS, min(SCS, L - i * SCS)) for i in range((L + SCS - 1) // SCS)]
SHARD = L // 4       # 576 pixels per rank
HCH = SHARD // 2     # 288-wide half-shard chunks
PADW = 50 * 51       # padded conv canvas (extra tail row for AP slack)
EPS = 1e-5
NCORES = 8

_cache = {}


def _build(vs, cvm):
    import concourse.bass as bass
    import concourse.bacc as bacc
    import concourse.tile as tile
    import concourse.mybir as mybir
    from contextlib import ExitStack

    f32 = mybir.dt.float32
    f32r = mybir.dt.float32r
    fp16 = mybir.dt.float16
    AF = mybir.ActivationFunctionType
    OP = mybir.AluOpType

    import concourse.hw_specs as hw_specs
    _orig_gat = hw_specs.get_activation_tables
    _KEEP = {"natural_log_exp_and_others", "silu_and_others"}

    def _patched_gat(arch):
        t = _orig_gat(arch)
        return {k: (v if k in _KEEP else set()) for k, v in t.items()}

    bacc.get_activation_tables = _patched_gat

    nc = bacc.Bacc("TRN2", target_bir_lowering=False, debug=False,
                   enable_asserts=True, num_devices=NCORES)

    def din(name, shape, dtype=f32):
        return nc.dram_tensor(name, shape, dtype, kind="ExternalInput").ap()

    x_shuf_d = din("x_shuf", (C_IN, L), fp16)
    x_my_d = din("x_my", (C_IN, SHARD))
    lhsT_cv_d = din("lhsT_cv", (99, 3, C_IN), fp16)   # conv, il-duplicated out
    lhsT_z_d = din("lhsT_z", (99, D), fp16)           # z half (dy=1 only)
    bias_cv_d = din("bias_cv", (C_IN, 1))             # silu bias, il-duplicated
    bias_z_d = din("bias_z", (D, 1))
    x_myr_d = din("x_my_r", (C_H, 4 * SHARD))
    lhsT_sq16_d = din("lhsT_sq16", (C_H, 1), fp16)
    lhsT_y1_d = din("lhsT_y1", (D, 1), fp16)
    lhsT_bc_d = din("lhsT_bc", (1, C_IN))
    lhsT_M2_d = din("lhsT_M2", (C_IN, K, C_IN), fp16)
    dtb2_d = din("dtb2", (C_IN, K))
    A2_d = din("A2", (C_IN, K, 2))
    lhsT_xbc_d = din("lhsT_xbc", (C_IN, K, 8), fp16)
    lhsT_ys_d = din("lhsT_ys", (C_IN, D), fp16)
    lhsT_ds_d = din("lhsT_ds", (C_IN, D), fp16)       # Ds init (k=2 psum)
    lhsT_op_d = din("lhsT_op", (D, C_H), fp16)        # includes /4 for sz RS
    lhsT_stf_d = din("lhsT_stf", (C_IN, 2), fp16)
    gamma_d = din("gamma", (C_IN, 1))
    beta_d = din("beta", (C_IN, 1))

    out_d = nc.dram_tensor("out_cf", (C_IN, SHARD), f32,
                           kind="ExternalOutput").ap()

    RG = [[0, 1, 2, 3], [4, 5, 6, 7]]

    with tile.TileContext(nc) as tc, ExitStack() as ctx:
        w_pool = ctx.enter_context(tc.tile_pool(name="weights", bufs=1))
        big = ctx.enter_context(tc.tile_pool(name="big", bufs=1))
        stg = ctx.enter_context(tc.tile_pool(name="stg", bufs=1))
        sml = ctx.enter_context(tc.tile_pool(name="sml", bufs=2))
        scn = ctx.enter_context(tc.tile_pool(name="scn", bufs=2))
        hpool = ctx.enter_context(tc.tile_pool(name="hpool", bufs=4))
        ps = ctx.enter_context(tc.tile_pool(name="ps", bufs=1, space="PSUM"))
        dram = ctx.enter_context(tc.tile_pool(name="dram", bufs=2, space="DRAM"))

        def wload(ap_d, shape, dtype=f32):
            t = w_pool.tile(list(shape), dtype, name=ap_d.tensor.name + "_sb")
            src = ap_d if ap_d.dtype == dtype else ap_d.bitcast(dtype)
            nc.sync.dma_start(t[:], src)
            return t

        x_my = wload(x_my_d, (C_IN, SHARD))
        lhsT_cv = wload(lhsT_cv_d, (99, 3, C_IN), fp16)
        lhsT_z = wload(lhsT_z_d, (99, D), fp16)
        bias_cv = wload(bias_cv_d, (C_IN, 1))
        bias_z = wload(bias_z_d, (D, 1))
        x_my_r = wload(x_myr_d, (C_H, 4 * SHARD))
        lhsT_sq16 = wload(lhsT_sq16_d, (C_H, 1), fp16)
        lhsT_y1 = wload(lhsT_y1_d, (D, 1), fp16)
        lhsT_bc = wload(lhsT_bc_d, (1, C_IN))
        lhsT_M2 = wload(lhsT_M2_d, (C_IN, K, C_IN), fp16)
        dtb2 = wload(dtb2_d, (C_IN, K))
        A2 = wload(A2_d, (C_IN, K, 2))
        lhsT_xbc = wload(lhsT_xbc_d, (C_IN, K, 8), fp16)
        lhsT_ys = wload(lhsT_ys_d, (C_IN, D), fp16)
        lhsT_ds = wload(lhsT_ds_d, (C_IN, D), fp16)
        lhsT_op = wload(lhsT_op_d, (D, C_H), fp16)
        lhsT_stf = wload(lhsT_stf_d, (C_IN, 2), fp16)
        gamma = wload(gamma_d, (C_IN, 1))
        beta = wload(beta_d, (C_IN, 1))

        # persistent tiles
        sr3 = big.tile([99, PADW], fp16)   # 3 shifted sr copies + 3 mr rows
        nc.vector.memset(sr3[:], 0.0)
        xs_il = big.tile([C_IN, L], fp16)    # duplicated-interleaved xs (rm)
        xs_cm = big.tile([C_IN, L], fp16)    # free-transposed xs_il (cm)
        outs_sh = big.tile([C_IN, SHARD], f32)
        prev_full = big.tile([C_H, L], fp16)
        mr_flat = big.tile([1, L], fp16)

        def ln_smalls(ps_m, ps_e, w, tagp):
            m_c = sml.tile([1, CS], f32, tag=tagp + "m", name="m_c")
            nc.scalar.copy(m_c[:, :w], ps_m[:, :w])
            m2_c = sml.tile([1, CS], f32, tag=tagp + "m2", name="m2_c")
            nc.scalar.square(m2_c[:, :w], m_c[:, :w])
            var_c = sml.tile([1, CS], f32, tag=tagp + "v", name="var_c")
            nc.vector.scalar_tensor_tensor(var_c[:, :w], ps_e[:, :w], EPS,
                                           m2_c[:, :w], OP.add, OP.subtract)
            lnv_c = sml.tile([1, CS], f32, tag=tagp + "m2", name="lnv_c")
            nc.scalar.activation(lnv_c[:, :w], var_c[:, :w], AF.Ln)
            r_c = sml.tile([1, CS], f32, tag=tagp + "r", name="r_c")
            nc.scalar.activation(r_c[:, :w], lnv_c[:, :w], AF.Exp, scale=-0.5)
            return r_c, m_c

        s_t = None
        prev_sh_ap = None
        for i in range(HEAD):
            # ---- stage input s (32, L) f32 ----
            chunk_sb = sml.tile([C_H, L], fp16, tag="s_cs", name="chunk_sb",
                                bufs=1)
            sz = big.tile([D, L], fp16, tag="szt", name="sz")
            y_rm = stg.tile([D, L], fp16, tag="srflat2", name="y_rm")
            nc.sync.dma_start(chunk_sb[:], x_shuf_d[32 * i:32 * (i + 1), :])
            if i == 0:
                s_t = chunk_sb[:]
            else:
                nc.vector.tensor_add(prev_full[:], prev_full[:], chunk_sb[:])
                s_t = prev_full[:]

            # ---- LN1 stats + sr = s*r (fp16) + mr row ----
            sq16 = stg.tile([C_H, L], fp16, tag="s16sq", name="sq16")
            sr_flat = stg.tile([C_H, L], fp16, tag="srflat", name="sr_flat")
            nc.scalar.square(sq16[:], s_t)
            for o, w in CHUNKS:
                ps_m = ps.tile([1, CS], f32, tag="sa", name="ps_m1")
                ps_e = ps.tile([1, CS], f32, tag="sd", name="ps_e1")
                nc.tensor.matmul(ps_m[:, :w], lhsT_sq16[:],
                                 s_t[:, o:o + w], start=True, stop=True)
                nc.tensor.matmul(ps_e[:, :w], lhsT_sq16[:],
                                 sq16[:, o:o + w], start=True, stop=True)
                r_c, m_c = ln_smalls(ps_m, ps_e, w, "sm")
                nc.vector.tensor_mul(mr_flat[:, o:o + w], m_c[:, :w],
                                     r_c[:, :w])
                ps_rr = ps.tile([C_H, CS], f32, tag="sb", name="ps_rr")
                nc.tensor.matmul(ps_rr[:, :w], lhsT_bc[0:1, 0:C_H],
                                 r_c[:, :w], start=True, stop=True)
                nc.vector.tensor_mul(sr_flat[:, o:o + w], s_t[:, o:o + w],
                                     ps_rr[:, :w])

            # ---- dx-shifted padded copies ----
            srf = sr_flat[:].rearrange("c (h w) -> c h w", h=48, w=48)
            mrf = mr_flat[:].rearrange("c (h w) -> c h w", h=48, w=48)
            for dx in range(3):
                b0 = max(0, dx - 1)
                b1 = min(47, 46 + dx)          # inclusive src col range
                wdt = b1 - b0 + 1
                base = 52 + b0 - dx
                dv = sr3[32 * dx:32 * dx + 32, base:base + 2400].rearrange(
                    "c (h w) -> c h w", h=48, w=50)[:, :, 0:wdt]
                nc.sync.dma_start(dv, srf[:, :, b0:b1 + 1])
                mv = sr3[96 + dx:97 + dx, base:base + 2400].rearrange(
                    "c (h w) -> c h w", h=48, w=50)[:, :, 0:wdt]
                nc.sync.dma_start(mv, mrf[:, :, b0:b1 + 1])

            # ---- conv/in_proj (il out) + z: accumulated matmuls ----
            h0 = 0
            while h0 < 48:
                nr = min(10, 48 - h0)
                wch = nr * 48
                ps_cv = ps.tile([C_IN, 480], f32, tag="sc", name="ps_cv")
                pv = ps_cv[:, :wch].rearrange("c (h w) -> c h w", h=nr, w=48)
                ps_z = ps.tile([D, 480], f32, tag="sb", name="ps_z")
                zv = ps_z[:, :wch].rearrange("c (h w) -> c h w", h=nr, w=48)
                for dy in range(3):
                    base = 1 + 50 * dy + 50 * h0
                    rv = sr3[:, base:base + 50 * nr].rearrange(
                        "c (h w) -> c h w", h=nr, w=50)[:, :, 0:48]
                    nc.tensor.matmul(pv, lhsT_cv[:, dy, :], rv,
                                     start=(dy == 0), stop=(dy == 2))
                    if dy == 1:
                        nc.tensor.matmul(zv, lhsT_z[:], rv,
                                         start=True, stop=True)
                nc.scalar.activation(xs_il[:, 48 * h0:48 * h0 + wch],
                                     ps_cv[:, :wch], AF.Silu, bias=bias_cv[:])
                nc.scalar.activation(sz[:, 48 * h0:48 * h0 + wch],
                                     ps_z[:, :wch], AF.Silu, bias=bias_z[:])
                h0 += nr

            # ---- cm layout: free-dim transpose of xs_il ----
            nc.vector.tensor_copy(
                xs_cm[:].rearrange("c (w h) -> c w h", h=48, w=48),
                xs_il[:].rearrange("c (h w) -> c w h", h=48, w=48))


            # ---- compact B/C (8 rows per k) -> fp16 (from the k-ordered xs) ----
            bc16 = stg.tile([8, K * L], fp16, tag="bc16", name="bc16")
            for k in range(K):
                xsk = xs_il if k in (0, 2) else xs_cm
                for o in range(0, L, 512):
                    sw = min(512, L - o)
                    ps_bc = ps.tile([8, 512], f32, tag="sd", name="ps_bc")
                    nc.tensor.matmul(ps_bc[:, :sw], lhsT_xbc[:, k, :],
                                     xsk[:, o:o + sw], start=True, stop=True)
                    nc.scalar.copy(bc16[:, k * L + o:k * L + o + sw],
                                   ps_bc[:, :sw])

            def expand_bc(k, gl):
                bsrc = bc16[2 * gl:2 * gl + 2, k * L:(k + 1) * L]
                csrc = bc16[4 + 2 * gl:6 + 2 * gl, k * L:(k + 1) * L]
                til = []
                for tag, src in (("Bil", bsrc), ("Cil", csrc)):
                    dstt = scn.tile([C_IN, L], fp16, tag=tag, name=tag,
                                    bufs=3)
                    nc.sync.dma_start(dstt[0:2, :], src)
                    rows = 2
                    while rows < C_IN:
                        nc.sync.dma_start(dstt[rows:2 * rows, :],
                                          dstt[0:rows, :])
                        rows *= 2
                    til.append(dstt)
                return til

            # ---- scans, k order: cm (1,3) then rm (2,0) ----
            y_cm = big.tile([D, L], fp16, tag="szycm", name="y_cm")
            rs_rm = [None, None]
            co_cm = None
            KORD = (1, 3, 2, 0)
            BCq = {}

            def get_bc(k):
                if k not in BCq:
                    BCq[k] = {gl: expand_bc(k, gl) for gl in range(2)}
                return BCq[k]

            get_bc(1)
            get_bc(3)
            for ki, k in enumerate(KORD):
                xs2 = xs_il if k in (0, 2) else xs_cm
                BC = get_bc(k)
                if ki + 1 < len(KORD):
                    get_bc(KORD[ki + 1])
                rev = k >= 2
                h_prev = {0: None, 1: None}
                corder = list(range(len(SCHUNKS)))
                if rev:
                    corder = corder[::-1]
                for ci in corder:
                    o, w = SCHUNKS[ci]
                    ps_dt = ps.tile([C_IN, SCS], f32, tag="pa", name="ps_dt")
                    for so in range(0, w, 512):
                        sw = min(512, w - so)
                        nc.tensor.matmul(ps_dt[:, so:so + sw], lhsT_M2[:, k, :],
                                         xs2[:, o + so:o + so + sw],
                                         start=True, stop=True)
                    e_ch = scn.tile([C_IN, SCS], fp16, tag="e_ch", name="e_ch", bufs=1)
                    nc.scalar.activation(e_ch[:, :w], ps_dt[:, :w], AF.Exp,
                                         bias=dtb2[:, k:k + 1])
                    dt2_c = scn.tile([C_IN, SCS], fp16, tag="dt2", name="dt2_c", bufs=1)
                    nc.scalar.activation(dt2_c[:, :w], e_ch[:, :w],
                                         AF.Ln, bias=1.0)
                    u2_c = scn.tile([C_IN, SCS], fp16, tag="u2", name="u2_c", bufs=1)
                    nc.vector.tensor_mul(u2_c[:, :w], dt2_c[:, :w],
                                         xs2[:, o:o + w])
                    subs = [(so, min(512, w - so)) for so in range(0, w, 512)]
                    ps_ys = {}
                    for gl in range(2):
                        dA = scn.tile([C_IN, SCS], f32, tag="dA", name="dA",
                                      bufs=1)
                        nc.scalar.activation(dA[:, :w], dt2_c[:, :w],
                                             AF.Exp, scale=A2[:, k, gl:gl + 1])
                        bB = scn.tile([C_IN, SCS], fp16, tag="bB", name="bB")
                        nc.vector.tensor_mul(bB[:, :w], u2_c[:, :w],
                                             BC[gl][0][:, o:o + w])
                        h_c = hpool.tile([C_IN, SCS], fp16, tag="h", name="h_c")
                        hp = h_prev[gl]
                        if not rev:
                            init = 0.0 if hp is None else hp[0][:, hp[1] - 1:hp[1]]
                            nc.vector.tensor_tensor_scan(
                                h_c[:, :w], dA[:, :w], bB[:, :w], init,
                                OP.mult, OP.add)
                        else:
                            init = 0.0 if hp is None else hp[0][:, 0:1]
                            nc.vector.tensor_tensor_scan(
                                h_c[:, :w][:, ::-1], dA[:, :w][:, ::-1],
                                bB[:, :w][:, ::-1], init, OP.mult, OP.add)
                        h_prev[gl] = (h_c, w)
                        hC = scn.tile([C_IN, SCS], fp16, tag="hC", name="hC")
                        nc.vector.tensor_mul(hC[:, :w], h_c[:, :w],
                                             BC[gl][1][:, o:o + w])
                        for so, sw in subs:
                            if gl == 0:
                                ps_ys[so] = ps.tile([D, 512], f32, tag="pd",
                                                    name="ps_y", bufs=2)
                                if k == 2:
                                    # fold the Ds*xs init into k=2's psum
                                    nc.tensor.matmul(
                                        ps_ys[so][:, :sw], lhsT_ds[:],
                                        xs_il[:, o + so:o + so + sw],
                                        start=True, stop=False,
                                        skip_group_check=True)
                            nc.tensor.matmul(ps_ys[so][:, :sw], lhsT_ys[:],
                                             hC[:, so:so + sw],
                                             start=(gl == 0 and k != 2),
                                             stop=(gl == 1),
                                             skip_group_check=True)
                    for so, sw in subs:
                        go = o + so
                        if k in (1, 2):
                            dst = y_cm if k == 1 else y_rm
                            nc.scalar.copy(dst[:, go:go + sw],
                                           ps_ys[so][:, :sw])
                        else:
                            dst = y_cm if k == 3 else y_rm
                            nc.vector.tensor_add(dst[:, go:go + sw],
                                                 dst[:, go:go + sw],
                                                 ps_ys[so][:, :sw])
                if k == 0:
                    ri = dram.tile([4 * D, SHARD], fp16,
                                   tag="rs_rm_in", name="ri", bufs=2)
                    ro = dram.tile([D, SHARD], fp16,
                                   tag="rs_rm_out", name="ro", bufs=2)
                    for r in range(4):
                        nc.sync.dma_start(
                            ri[D * r:D * (r + 1), :],
                            y_rm[:, r * SHARD:(r + 1) * SHARD])
                    nc.gpsimd.collective_compute(
                        "ReduceScatter", OP.add, replica_groups=RG,
                        ins=[ri[:].opt()], outs=[ro[:].opt()])
                    rs_rm[0] = ro
                if k == 3:
                    # cm pair complete: transpose then RS (hidden under rm)
                    y_cmg = stg.tile([D, L], fp16, tag="s16sq", name="y_cmg")
                    nc.vector.tensor_copy(
                        y_cmg[:].rearrange("c (h w) -> c h w", h=48, w=48),
                        y_cm[:].rearrange("c (w h) -> c h w", h=48, w=48))
                    ci_cm = dram.tile([4 * C_IN, SHARD], fp16,
                                      tag="rs_cm_in", name="ci_cm", bufs=2)
                    co_cm = dram.tile([C_IN, SHARD], fp16, tag="rs_cm_out",
                                      name="co_cm", bufs=2)
                    for r in range(4):
                        nc.sync.dma_start(
                            ci_cm[C_IN * r:C_IN * r + D, :],
                            y_cmg[:, r * SHARD:(r + 1) * SHARD])
                        nc.sync.dma_start(
                            ci_cm[C_IN * r + D:C_IN * (r + 1), :],
                            sz[:, r * SHARD:(r + 1) * SHARD])
                    nc.gpsimd.collective_compute(
                        "ReduceScatter", OP.add, replica_groups=RG,
                        ins=[ci_cm[:].opt()], outs=[co_cm[:].opt()])

            # ---- sharded tail ----
            cm_sh = sml.tile([D, SHARD], fp16, tag="cm_sh", name="cm_sh")
            nc.sync.dma_start(cm_sh[:], co_cm[0:D, :])
            sz_sh = sml.tile([D, SHARD], fp16, tag="sz_sh", name="sz_sh")
            nc.sync.dma_start(sz_sh[:], co_cm[D:C_IN, :])
            rm_sh = sml.tile([D, SHARD], fp16, tag="rm_sh", name="rm_sh")
            nc.sync.dma_start(rm_sh[:], rs_rm[0][:])
            if i == 0:
                s_sh = x_my_r[:, 0:SHARD]
            else:
                s_sh_t = sml.tile([C_H, SHARD], f32, tag="s_sh", name="s_sh")
                nc.vector.tensor_add(s_sh_t[:], prev_sh_ap,
                                     x_my_r[:, i * SHARD:(i + 1) * SHARD])
                s_sh = s_sh_t[:]
            y_sh = sml.tile([D, SHARD], fp16, tag="y_sh", name="y_sh")
            nc.vector.tensor_add(y_sh[:], rm_sh[:], cm_sh[:])
            ysq_t = sml.tile([D, SHARD], fp16, tag="ysqt", name="ysq_t")
            nc.scalar.square(ysq_t[:], y_sh[:])
            prev_sh = sml.tile([C_H, SHARD], f32, tag="prevsh",
                               name="prev_sh")
            for hh in range(2):
                o = hh * HCH
                ps_m2 = ps.tile([1, HCH], f32, tag="sa", name="ps_m2")
                ps_e2 = ps.tile([1, HCH], f32, tag="sd", name="ps_e2")
                nc.tensor.matmul(ps_m2[:], lhsT_y1[:], y_sh[:, o:o + HCH],
                                 start=True, stop=True)
                nc.tensor.matmul(ps_e2[:], lhsT_y1[:], ysq_t[:, o:o + HCH],
                                 start=True, stop=True)
                r_c, m_c = ln_smalls(ps_m2, ps_e2, HCH, "sm")
                ps_mb = ps.tile([D, HCH], f32, tag="sb", name="ps_mb")
                nc.tensor.matmul(ps_mb[:], lhsT_bc[0:1, 0:D], m_c[:, :HCH],
                                 start=True, stop=True)
                ps_rb = ps.tile([C_H, HCH], f32, tag="sc", name="ps_rb")
                nc.tensor.matmul(ps_rb[:], lhsT_bc[0:1, 0:C_H], r_c[:, :HCH],
                                 start=True, stop=True)
                ym = sml.tile([D, HCH], f32, tag="ym", name="ym")
                nc.vector.tensor_sub(ym[:], y_sh[:, o:o + HCH], ps_mb[:])
                ysz = sml.tile([D, HCH], fp16, tag="ysz", name="ysz")
                nc.vector.tensor_mul(ysz[:], ym[:], sz_sh[:, o:o + HCH])
                ps_op = ps.tile([C_H, HCH], f32, tag="pd", name="ps_op", bufs=2)
                nc.tensor.matmul(ps_op[:], lhsT_op[:], ysz[:],
                                 start=True, stop=True)
                op_sb = sml.tile([C_H, HCH], f32, tag="op_sb", name="op_sb")
                nc.scalar.copy(op_sb[:], ps_op[:])
                t_c = sml.tile([C_H, HCH], f32, tag="t_c", name="t_c")
                nc.vector.tensor_mul(t_c[:], op_sb[:], ps_rb[:])
                nc.vector.scalar_tensor_tensor(
                    prev_sh[:, o:o + HCH],
                    s_sh[:, o:o + HCH], 1.0 + vs, t_c[:], OP.mult, OP.add)
            nc.sync.dma_start(outs_sh[32 * i:32 * (i + 1), :], prev_sh[:])
            prev_sh_ap = prev_sh[:]

            if i < HEAD - 1:
                prev16 = sml.tile([C_H, SHARD], fp16, tag="prev16",
                                  name="prev16")
                nc.scalar.copy(prev16[:], prev_sh[:])
                agi = dram.tile([C_H, SHARD], fp16, tag="ag_in", name="agi",
                                bufs=2)
                ago = dram.tile([4 * C_H, SHARD], fp16, tag="ag_out",
                                name="ago", bufs=2)
                nc.sync.dma_start(agi[:], prev16[:])
                nc.gpsimd.collective_compute(
                    "AllGather", OP.bypass, replica_groups=RG,
                    ins=[agi[:].opt()], outs=[ago[:].opt()])
                for r in range(4):
                    nc.sync.dma_start(
                        prev_full[:, r * SHARD:(r + 1) * SHARD],
                        ago[C_H * r:C_H * (r + 1), :])

        # ---- final: x_res = cvm*x_my + outs_sh (shard); LN over 128 ch ----
        xres = stg.tile([C_IN, SHARD], f32, tag="s16sq", name="xres")
        nc.vector.scalar_tensor_tensor(xres[:], x_my[:], cvm,
                                       outs_sh[:], OP.mult, OP.add)
        x16 = stg.tile([C_IN, SHARD], fp16, tag="srflat", name="x16")
        nc.scalar.copy(x16[:], xres[:])
        xsq = stg.tile([C_IN, SHARD], fp16, tag="bc16", name="xsq")
        nc.scalar.square(xsq[:], xres[:])
        out_sb = stg.tile([C_IN, SHARD], f32, tag="out_sb", name="out_sb")
        for hh in range(2):
            o = hh * HCH
            ps_m = ps.tile([1, HCH], f32, tag="sa", name="ps_m3")
            ps_e = ps.tile([1, HCH], f32, tag="sb", name="ps_e3")
            nc.tensor.matmul(ps_m[:], lhsT_stf[:, 0:1], x16[:, o:o + HCH],
                             start=True, stop=True)
            nc.tensor.matmul(ps_e[:], lhsT_stf[:, 1:2], xsq[:, o:o + HCH],
                             start=True, stop=True)
            r_c, m_c = ln_smalls(ps_m, ps_e, HCH, "sm")
            mr_c = sml.tile([1, HCH], f32, tag="smv", name="mr_c3")
            nc.vector.tensor_mul(mr_c[:], m_c[:, :HCH], r_c[:, :HCH])
            ps_ra = ps.tile([C_IN, HCH], f32, tag="sc", name="ps_ra3")
            nc.tensor.matmul(ps_ra[:], lhsT_bc[:], r_c[:, :HCH],
                             start=True, stop=True)
            ps_rb = ps.tile([C_IN, HCH], f32, tag="pd", name="ps_rb3", bufs=2)
            nc.tensor.matmul(ps_rb[:], lhsT_bc[:], mr_c[:],
                             start=True, stop=True)
            nc.vector.tensor_mul(out_sb[:, o:o + HCH], xres[:, o:o + HCH],
                                 ps_ra[:])
            nc.vector.tensor_sub(out_sb[:, o:o + HCH], out_sb[:, o:o + HCH],
                                 ps_rb[:])
            nc.vector.tensor_scalar(out_sb[:, o:o + HCH],
                                    out_sb[:, o:o + HCH],
                                    gamma[:], beta[:], OP.mult, OP.add)
        nc.sync.dma_start(out_d[:], out_sb[:])

    nc.compile()
    return nc


def _host_prep(inputs):
    """Build per-core input maps from full inputs."""
    fp16 = np.float16
    x = np.asarray(inputs["x"], np.float32)
    ln1_w = np.asarray(inputs["ln1_w"], np.float32)
    ln1_b = np.asarray(inputs["ln1_b"], np.float32)
    in_proj_w = np.asarray(inputs["in_proj_w"], np.float32)
    conv_w = np.asarray(inputs["conv_w"], np.float32)
    conv_b = np.asarray(inputs["conv_b"], np.float32)
    x_proj_w = np.asarray(inputs["x_proj_w"], np.float32)
    dt_proj_w = np.asarray(inputs["dt_proj_w"], np.float32)
    dt_proj_b = np.asarray(inputs["dt_proj_b"], np.float32)
    A_logs = np.asarray(inputs["A_logs"], np.float32)
    Ds = np.asarray(inputs["Ds"], np.float32)
    out_norm_w = np.asarray(inputs["out_norm_w"], np.float32)
    out_norm_b = np.asarray(inputs["out_norm_b"], np.float32)
    out_proj_w = np.asarray(inputs["out_proj_w"], np.float32)
    final_ln_w = np.asarray(inputs["final_ln_w"], np.float32)
    final_ln_b = np.asarray(inputs["final_ln_b"], np.float32)
    assert not np.any(out_norm_b), "out_norm_b must be zero (folded)"

    Wxx = (in_proj_w[0:D] * ln1_w[None, :]).astype(np.float32)    # (64, 32)
    Wz = (in_proj_w[D:C_IN] * ln1_w[None, :]).astype(np.float32)  # (64, 32)
    bias_ip = (in_proj_w @ ln1_b).astype(np.float32)              # (128,)
    w9 = np.ascontiguousarray(
        conv_w[:, :, 0, :].transpose(2, 0, 1).reshape(D, 9))      # (64, 9)
    colsum_xx = Wxx.sum(1)
    colsum_z = Wz.sum(1)

    # conv lhsT with duplicated-interleaved output: out col p = 2d+half
    lhsT_cv = np.zeros((99, 3, C_IN), fp16)
    for dy in range(3):
        for dx in range(3):
            tap = 3 * dy + dx
            blk = (w9[:, tap][None, :] * Wxx.T)          # (32c, 64d)
            cor = (-w9[:, tap] * colsum_xx)              # (64d,)
            for half in range(2):
                lhsT_cv[32 * dx:32 * dx + 32, dy, half::2] = blk.astype(fp16)
                lhsT_cv[96 + dx, dy, half::2] = cor.astype(fp16)
    lhsT_z = np.zeros((99, D), fp16)
    lhsT_z[32:64, :] = Wz.T.astype(fp16)
    lhsT_z[97, :] = (-colsum_z).astype(fp16)

    bias_cv = np.zeros((C_IN, 1), np.float32)
    bcv = conv_b + w9.sum(1) * bias_ip[0:D]
    bias_cv[0::2, 0] = bcv
    bias_cv[1::2, 0] = bcv
    bias_z = bias_ip[D:C_IN].reshape(D, 1).astype(np.float32)

    lhsT_sq16 = np.full((C_H, 1), 1.0 / C_H, fp16)
    lhsT_y1 = np.full((D, 1), 1.0 / D, fp16)
    lhsT_bc = np.ones((1, C_IN), np.float32)

    # dt projection: contraction over il rows (2c+half -> /2), out il cols
    M = np.einsum("kdr,krc->kdc", dt_proj_w, x_proj_w[:, :DT_RANK, :])
    lhsT_M2 = np.zeros((C_IN, K, C_IN), fp16)
    dtb2 = np.zeros((C_IN, K), np.float32)
    for k in range(K):
        mh = (M[k].T / 2.0).astype(fp16)                 # (32?? c, d) -> (64c, 64d)
        for half_in in range(2):
            for half_out in range(2):
                lhsT_M2[half_in::2, k, half_out::2] = mh
        dtb2[0::2, k] = dt_proj_b[k, :]
        dtb2[1::2, k] = dt_proj_b[k, :]

    A = -np.exp(A_logs)                                  # (K, 64, 16)
    Ds_q = (Ds.sum(0) / 4.0).astype(np.float32)          # (64,)
    # out_proj lhsT; includes out_norm gamma fold and the /4 for the sz RS
    W_op = (out_proj_w * out_norm_w[None, :]) / 4.0
    lhsT_op = np.ascontiguousarray(W_op.T).astype(fp16)  # (64, 32)

    lhsT_ys = np.zeros((C_IN, D), fp16)
    lhsT_ds = np.zeros((C_IN, D), fp16)
    for d in range(D):
        lhsT_ys[2 * d, d] = 1.0
        lhsT_ys[2 * d + 1, d] = 1.0
        lhsT_ds[2 * d, d] = Ds_q[d] / 2.0
        lhsT_ds[2 * d + 1, d] = Ds_q[d] / 2.0
    lhsT_stf = np.zeros((C_IN, 2), fp16)
    lhsT_stf[:, 0] = 1.0 / C_IN
    lhsT_stf[:, 1] = 1.0 / C_IN

    common = {
        "lhsT_cv": lhsT_cv, "lhsT_z": lhsT_z,
        "bias_cv": bias_cv, "bias_z": bias_z,
        "lhsT_sq16": lhsT_sq16,
        "lhsT_y1": lhsT_y1, "lhsT_bc": lhsT_bc,
        "lhsT_M2": lhsT_M2, "dtb2": dtb2,
        "lhsT_ys": lhsT_ys, "lhsT_ds": lhsT_ds,
        "lhsT_op": lhsT_op,
        "lhsT_stf": lhsT_stf,
        "gamma": final_ln_w.reshape(C_IN, 1),
        "beta": final_ln_b.reshape(C_IN, 1),
    }

    g = HEAD
    cg = C_IN // HEAD
    per_b = []
    per_b32 = []
    for b in range(B):
        xs = x[b].reshape(H, W, g, cg).transpose(0, 1, 3, 2).reshape(L, C_IN)
        cf = np.ascontiguousarray(xs.T)
        per_b32.append(cf)
        per_b.append(cf.astype(fp16))  # (128, L)

    in_maps = []
    for c in range(NCORES):
        b, nh = c // 4, c % 4
        A2 = np.zeros((C_IN, K, 2), np.float32)
        lhsT_xbc = np.zeros((C_IN, K, 8), fp16)
        for k in range(K):
            for gl in range(2):
                for half in range(2):
                    n = 4 * nh + 2 * gl + half
                    for d in range(D):
                        A2[2 * d + half, k, gl] = A[k, d, n]
            for j in range(4):
                rowb = (x_proj_w[k, DT_RANK + (4 * nh + j), :] / 2.0)
                rowc = (x_proj_w[k, DT_RANK + N + (4 * nh + j), :] / 2.0)
                for half in range(2):
                    lhsT_xbc[half::2, k, j] = rowb.astype(fp16)
                    lhsT_xbc[half::2, k, 4 + j] = rowc.astype(fp16)
        x_shuf_cf = per_b[b]
        x_my = np.ascontiguousarray(
            per_b32[b][:, nh * SHARD:(nh + 1) * SHARD])
        x_my_r = np.zeros((C_H, 4 * SHARD), np.float32)
        for i in range(HEAD):
            x_my_r[:, i * SHARD:(i + 1) * SHARD] = x_my[32 * i:32 * (i + 1)]
        in_maps.append(dict(common, x_shuf=x_shuf_cf, x_my=x_my,
                            x_my_r=x_my_r, A2=A2, lhsT_xbc=lhsT_xbc))
    vs = float(np.asarray(inputs["vss_skip"]).ravel()[0])
    cvm = float(np.asarray(inputs["cvm_skip"]).ravel()[0])
    return in_maps, vs, cvm


def kernel(**inputs) -> np.ndarray:
    from concourse.bass_utils import run_bass_kernel_spmd

    in_maps, vs, cvm = _host_prep(inputs)
    key = (vs, cvm)
    if key not in _cache:
        _cache[key] = _build(vs, cvm)
    nc = _cache[key]
    res = run_bass_kernel_spmd(nc, in_maps, core_ids=list(range(NCORES)))
    out = np.zeros((B, H, W, C_IN), np.float32)
    for b in range(B):
        full = np.zeros((C_IN, L), np.float32)
        for r in range(4):
            full[:, r * SHARD:(r + 1) * SHARD] = \
                res.results[4 * b + r]["out_cf"]
        out[b] = full.T.reshape(H, W, C_IN)
    return out



# revision 18
# speedup vs baseline: 1.2844x; 1.2844x over previous
"""Trainium2 Bass kernel for nn_CascadedVMambaBlock (v2).

Sharding: 8 cores; core c = (b, nh) with b = c//4, nh = c%4.
Each core processes sample b with state-dim slice n in [4nh, 4nh+4)
for ALL 4 scan directions k. Per-head combine is a ReduceScatter over
the 4-core b-group (each rank keeps an L-shard of 576 pixels), the
out_norm/out_proj tail runs sharded, and an AllGather rebuilds the
full next-stage input.

Layouts: channels-first (channels on partitions, L = 2304 free).
Scan lanes are INTERLEAVED: partition p = 2*d + half covers channel
d = p//2, state n = 4*nh + 2*gl + half.

v2 structure (vs v1):
- B/C row-expansion: the x_proj matmul emits a [128,512] seed whose
  output rows repeat the 8-row (B0h0,B0h1,B1h0,B1h1,C0..C1..) pattern
  16x (PE cost is free-dim only), one scalar copy -> fp16 seed tile,
  then per (gl,tensor) just 3 chained SBUF DMAs (32 rows gathered,
  then 2 log-doublings) instead of a 7-deep chain.
- Scan-prep (dt matmul, softplus, u2, dA, B/C) is computed full-L per
  direction k, 1-2 directions ahead, so the DVE runs the scan phase
  as one op per (k,gl) with no intra-k chaining.
- xs_cm is written directly from conv PSUM by a second strided silu;
  cm ys chunks write the h-major transpose (y_cmg) directly.
- KORD = (0,2,1,3): the rm ReduceScatter is issued mid-head and mostly
  hidden; only the cm ReduceScatter (y + sz piggyback) is exposed.
"""
import numpy as np

HEAD, C_IN, C_H = 4, 128, 32
D, N, K, DT_RANK = 64, 16, 4, 2
B, H, W = 2, 48, 48
L = H * W            # 2304
CS = 480             # LN1 chunk (10 image rows -> canvas-aligned)
LNCH = [(i * CS, min(CS, L - i * CS)) for i in range((L + CS - 1) // CS)]
SHARD = L // 4       # 576 pixels per rank
HCH = SHARD // 2     # 288-wide half-shard chunks
PADW = 50 * 51       # padded conv canvas (extra tail row for AP slack)
EPS = 1e-5
NCORES = 8
KORD = (0, 2, 1, 3)

_cache = {}


DEBUG = False


def _build(vs, cvm):
    import concourse.bass as bass
    import concourse.bacc as bacc
    import concourse.tile as tile
    import concourse.mybir as mybir
    from contextlib import ExitStack

    f32 = mybir.dt.float32
    fp16 = mybir.dt.float16
    AF = mybir.ActivationFunctionType
    OP = mybir.AluOpType

    import concourse.hw_specs as hw_specs
    _orig_gat = hw_specs.get_activation_tables
    _KEEP = {"natural_log_exp_and_others", "silu_and_others"}

    def _patched_gat(arch):
        t = _orig_gat(arch)
        return {k: (v if k in _KEEP else set()) for k, v in t.items()}

    bacc.get_activation_tables = _patched_gat

    nc = bacc.Bacc("TRN2", target_bir_lowering=False, debug=False,
                   enable_asserts=True, num_devices=NCORES)

    def din(name, shape, dtype=f32):
        return nc.dram_tensor(name, shape, dtype, kind="ExternalInput").ap()

    x_shuf_d = din("x_shuf", (C_IN, L), fp16)
    x_my_d = din("x_my", (C_IN, SHARD))
    lhsT_cv_d = din("lhsT_cv", (99, 3, C_IN), fp16)   # conv, il-duplicated out
    lhsT_z_d = din("lhsT_z", (99, D), fp16)           # z half (dy=1 only)
    bias_cv_d = din("bias_cv", (C_IN, 1))             # silu bias, il-duplicated
    bias_z_d = din("bias_z", (D, 1))
    x_myr_d = din("x_my_r", (C_H, 4 * SHARD), fp16)
    lhsT_sq16_d = din("lhsT_sq16", (C_H, 1), fp16)
    lhsT_y1_d = din("lhsT_y1", (D, 1), fp16)
    lhsT_bc_d = din("lhsT_bc", (1, C_IN))
    lhsT_M2_d = din("lhsT_M2", (C_IN, K, C_IN), fp16)
    dtb2_d = din("dtb2", (C_IN, K))
    A2_d = din("A2", (C_IN, K, 2))
    lhsT_seed_d = din("lhsT_seed", (C_IN, K, C_IN), fp16)
    lhsT_ys_d = din("lhsT_ys", (C_IN, D), fp16)
    lhsT_ds_d = din("lhsT_ds", (C_IN, D), fp16)       # Ds init (k=0 psum)
    lhsT_op_d = din("lhsT_op", (D, C_H), fp16)
    lhsT_stf_d = din("lhsT_stf", (C_IN, 2), fp16)
    gamma_d = din("gamma", (C_IN, 1))
    beta_d = din("beta", (C_IN, 1))

    out_d = nc.dram_tensor("out_cf", (C_IN, SHARD), f32,
                           kind="ExternalOutput").ap()
    dbg_d = {}
    if DEBUG:
        for nm, shp in (("xs_il", (C_IN, L)), ("xs_cm", (C_IN, L)),
                        ("sz", (D, L)), ("seed0", (C_IN, L)),
                        ("B0k0", (C_IN, L)), ("C1k0", (C_IN, L)),
                        ("dt2k0", (C_IN, L)), ("u2k0", (C_IN, L)),
                        ("dA0k0", (C_IN, L)), ("h0k0", (C_IN, L)),
                        ("y_rm", (D, L)), ("y_cmg", (D, L)),
                        ("srctr", (C_H, PADW)), ("mrctr", (1, PADW))):
            dbg_d[nm] = nc.dram_tensor("dbg_" + nm, shp, fp16,
                                       kind="ExternalOutput").ap()

    RG = [[0, 1, 2, 3], [4, 5, 6, 7]]

    with tile.TileContext(nc) as tc, ExitStack() as ctx:
        w_pool = ctx.enter_context(tc.tile_pool(name="weights", bufs=1))
        big = ctx.enter_context(tc.tile_pool(name="big", bufs=1))
        stg = ctx.enter_context(tc.tile_pool(name="stg", bufs=1))
        sml = ctx.enter_context(tc.tile_pool(name="sml", bufs=2))
        scn = ctx.enter_context(tc.tile_pool(name="scn", bufs=2))
        ps = ctx.enter_context(tc.tile_pool(name="ps", bufs=1, space="PSUM"))
        dram = ctx.enter_context(tc.tile_pool(name="dram", bufs=2, space="DRAM"))

        def wload(ap_d, shape, dtype=f32):
            t = w_pool.tile(list(shape), dtype, name=ap_d.tensor.name + "_sb")
            src = ap_d if ap_d.dtype == dtype else ap_d.bitcast(dtype)
            nc.sync.dma_start(t[:], src)
            return t

        x_my = wload(x_my_d, (C_IN, SHARD))
        lhsT_cv = wload(lhsT_cv_d, (99, 3, C_IN), fp16)
        lhsT_z = wload(lhsT_z_d, (99, D), fp16)
        bias_cv = wload(bias_cv_d, (C_IN, 1))
        bias_z = wload(bias_z_d, (D, 1))
        x_my_r = wload(x_myr_d, (C_H, 4 * SHARD), fp16)
        lhsT_sq16 = wload(lhsT_sq16_d, (C_H, 1), fp16)
        lhsT_y1 = wload(lhsT_y1_d, (D, 1), fp16)
        lhsT_bc = wload(lhsT_bc_d, (1, C_IN))
        lhsT_M2 = wload(lhsT_M2_d, (C_IN, K, C_IN), fp16)
        dtb2 = wload(dtb2_d, (C_IN, K))
        A2 = wload(A2_d, (C_IN, K, 2))
        lhsT_seed = wload(lhsT_seed_d, (C_IN, K, C_IN), fp16)
        lhsT_ys = wload(lhsT_ys_d, (C_IN, D), fp16)
        lhsT_ds = wload(lhsT_ds_d, (C_IN, D), fp16)
        lhsT_op = wload(lhsT_op_d, (D, C_H), fp16)
        lhsT_stf = wload(lhsT_stf_d, (C_IN, 2), fp16)
        gamma = wload(gamma_d, (C_IN, 1))
        beta = wload(beta_d, (C_IN, 1))

        # persistent tiles
        # canvas rows: 0:32 sr(dx0), 32:64 sr(center/dx1), 64:96 sr(dx2),
        #              96 mr(center/dx1), 97 mr(dx0), 98 mr(dx2)
        # (mr center must sit on a quarter-aligned partition for DVE writes)
        sr3 = big.tile([99, PADW], fp16)
        nc.vector.memset(sr3[:], 0.0)
        xs_il = big.tile([C_IN, L], fp16)    # duplicated-interleaved xs (rm)
        xs_cm = big.tile([C_IN, L], fp16)    # col-major layout (from conv)
        outs_sh = big.tile([C_IN, SHARD], f32)
        prev_full = big.tile([C_H, L], fp16)
        y_rm = big.tile([D, L], fp16)
        y_cmg = big.tile([D, L], fp16)       # cm result, already h-major
        sz = big.tile([D, L], fp16)          # silu(z), full L

        # canvas views
        CTR = sr3[32:64, :]

        def cview(rows, j, nr):
            # canvas view writing sr[h, w] at position 51 + 50h + w: the
            # CENTER region is left-shifted by one col (R_1[h, b] = sr[h, b+1])
            base = 51 + 500 * j
            return rows[:, base:base + 50 * nr].rearrange(
                "c (h w) -> c h w", h=nr, w=50)[:, :, 0:48]

        def ln_smalls(ps_me, w, tagp):
            # ps_me psum: row 0 = mean, row 32 = E[x^2]
            m2_c = sml.tile([1, CS], f32, tag=tagp + "m2", name="m2_c")
            nc.scalar.square(m2_c[:, :w], ps_me[0:1, :w])
            var_c = sml.tile([1, CS], f32, tag=tagp + "v", name="var_c")
            nc.vector.scalar_tensor_tensor(var_c[:, :w], ps_me[32:33, :w],
                                           EPS, m2_c[:, :w], OP.add,
                                           OP.subtract)
            lnv_c = sml.tile([1, CS], f32, tag=tagp + "m2", name="lnv_c")
            nc.scalar.activation(lnv_c[:, :w], var_c[:, :w], AF.Ln)
            r_c = sml.tile([1, CS], f32, tag=tagp + "r", name="r_c")
            nc.scalar.activation(r_c[:, :w], lnv_c[:, :w], AF.Exp, scale=-0.5)
            return r_c

        s_t = None
        prev_sh_ap = None
        for i in range(HEAD):
            # ---- stage input s (32, L) ----
            chunk_sb = sml.tile([C_H, L], fp16, tag="s_cs", name="chunk_sb",
                                bufs=1)
            nc.sync.dma_start(chunk_sb[:], x_shuf_d[32 * i:32 * (i + 1), :])
            if i == 0:
                s_t = chunk_sb[:]
            else:
                nc.vector.tensor_add(prev_full[:], prev_full[:], chunk_sb[:])
                s_t = prev_full[:]

            # ---- LN1: stats + sr/mr written into canvas center ----
            for ci, (o, w) in enumerate(LNCH):
                nr = w // 48
                sq_c = sml.tile([C_H, CS], fp16, tag="sqc", name="sq_c")
                nc.scalar.square(sq_c[:, :w], s_t[:, o:o + w])
                ps_me = ps.tile([33, 512], f32, tag="st", name="ps_me",
                                bufs=1)
                nc.tensor.matmul(ps_me[0:1, :w], lhsT_sq16[:],
                                 s_t[:, o:o + w], start=True, stop=True)
                nc.tensor.matmul(ps_me[32:33, :w], lhsT_sq16[:],
                                 sq_c[:, :w], start=True, stop=True)
                r_c = ln_smalls(ps_me, w, "sm")
                # mr -> canvas row 96 (center), strided over rows
                nc.vector.tensor_mul(
                    cview(sr3[96:97, :], ci, nr), ps_me[0:1, :w].rearrange(
                        "c (h w) -> c h w", h=nr, w=48),
                    r_c[:, :w].rearrange("c (h w) -> c h w", h=nr, w=48))
                ps_rr = ps.tile([C_IN, 512], f32, tag="sp", name="ps_rr",
                                bufs=2)
                nc.tensor.matmul(ps_rr[0:C_H, :w], lhsT_bc[0:1, 0:C_H],
                                 r_c[:, :w], start=True, stop=True)
                nc.vector.tensor_mul(
                    cview(CTR, ci, nr),
                    s_t[:, o:o + w].rearrange("c (h w) -> c h w", h=nr, w=48),
                    ps_rr[0:C_H, :w].rearrange("c (h w) -> c h w", h=nr, w=48))

            # ---- dx-shifted copies (center -> dx0/dx2 regions) ----
            ctr_flat = sr3[32:64, 51:51 + 2400].rearrange(
                "c (h w) -> c h w", h=48, w=50)
            mr_flat = sr3[96:97, 51:51 + 2400].rearrange(
                "c (h w) -> c h w", h=48, w=50)
            for dx in (0, 2):
                b0 = max(0, dx - 1)
                b1 = min(47, 46 + dx)          # inclusive src col range
                wdt = b1 - b0 + 1
                base = 52 + b0 - dx
                mrrow = 97 if dx == 0 else 98
                dv = sr3[32 * dx:32 * dx + 32, base:base + 2400].rearrange(
                    "c (h w) -> c h w", h=48, w=50)[:, :, 0:wdt]
                nc.sync.dma_start(dv, ctr_flat[:, :, b0:b1 + 1])
                mv = sr3[mrrow:mrrow + 1, base:base + 2400].rearrange(
                    "c (h w) -> c h w", h=48, w=50)[:, :, 0:wdt]
                nc.sync.dma_start(mv, mr_flat[:, :, b0:b1 + 1])

            # ---- conv/in_proj: accumulated matmuls; dual silu out ----
            h0 = 0
            while h0 < 48:
                nr = min(10, 48 - h0)
                wch = nr * 48
                ps_cv = ps.tile([C_IN, 512], f32, tag="cv", name="ps_cv",
                                bufs=1)
                pv = ps_cv[:, :wch].rearrange("c (h w) -> c h w", h=nr, w=48)
                ps_z = ps.tile([D, 512], f32, tag="ys", name="ps_z", bufs=2)
                zv = ps_z[:, :wch].rearrange("c (h w) -> c h w", h=nr, w=48)
                for dy in range(3):
                    base = 1 + 50 * dy + 50 * h0
                    rv = sr3[:, base:base + 50 * nr].rearrange(
                        "c (h w) -> c h w", h=nr, w=50)[:, :, 0:48]
                    nc.tensor.matmul(pv, lhsT_cv[:, dy, :], rv,
                                     start=(dy == 0), stop=(dy == 2))
                    if dy == 1:
                        nc.tensor.matmul(zv, lhsT_z[:], rv,
                                         start=True, stop=True)
                nc.scalar.activation(sz[:, 48 * h0:48 * h0 + wch],
                                     ps_z[:, :wch], AF.Silu, bias=bias_z[:])
                nc.scalar.activation(xs_il[:, 48 * h0:48 * h0 + wch],
                                     ps_cv[:, :wch], AF.Silu, bias=bias_cv[:])
                # second silu: write col-major layout directly
                cm_dst = xs_cm[:].rearrange(
                    "c (w h) -> c w h", w=48, h=48)[:, :, h0:h0 + nr]
                nc.scalar.activation(
                    cm_dst, ps_cv[:, :wch].rearrange(
                        "c (h w) -> c w h", h=nr, w=48),
                    AF.Silu, bias=bias_cv[:])
                h0 += nr

            # ---- scan preps + scans ----
            def prep(k):
                xs2 = xs_il if k in (0, 2) else xs_cm
                ech = scn.tile([C_IN, L], fp16, tag="ech", name="ech", bufs=1)
                for o in range(0, L, 1024):
                    cw = min(1024, L - o)
                    ps_dt = ps.tile([C_IN, 1024], f32, tag="pdt",
                                    name="ps_dt", bufs=1)
                    for so in range(0, cw, 512):
                        sw = min(512, cw - so)
                        nc.tensor.matmul(ps_dt[:, so:so + sw],
                                         lhsT_M2[:, k, :],
                                         xs2[:, o + so:o + so + sw],
                                         start=True, stop=True)
                    nc.scalar.activation(ech[:, o:o + cw], ps_dt[:, :cw],
                                         AF.Exp, bias=dtb2[:, k:k + 1])
                dt2 = scn.tile([C_IN, L], fp16, tag="dt2", name="dt2", bufs=2)
                nc.scalar.activation(dt2[:], ech[:], AF.Ln, bias=1.0)
                dAs = []
                for gl in range(2):
                    dA = scn.tile([C_IN, L], fp16, tag=f"dA{gl}", name="dA",
                                  bufs=2)
                    nc.scalar.activation(dA[:], dt2[:], AF.Exp,
                                         scale=A2[:, k, gl:gl + 1])
                    dAs.append(dA)
                seed = scn.tile([C_IN, L], fp16, tag="seed", name="seed",
                                bufs=1)
                for o in range(0, L, 512):
                    sw = min(512, L - o)
                    ps_sd = ps.tile([C_IN, 512], f32, tag="sp", name="ps_sd",
                                    bufs=2)
                    nc.tensor.matmul(ps_sd[:, :sw], lhsT_seed[:, k, :],
                                     xs2[:, o:o + sw], start=True, stop=True)
                    nc.scalar.copy(seed[:, o:o + sw], ps_sd[:, :sw])
                # expansion: 4 DMAs per (tensor, gl); SBUF APs may only
                # stride partitions in dim 0, so gather each parity separately
                BC = {}
                for ti, tag in enumerate(("B0", "B1", "C0", "C1")):
                    t0 = (ti % 2) * 2 + (ti // 2) * 4
                    dstt = scn.tile([C_IN, L], fp16, tag=tag, name=tag,
                                    bufs=2)
                    for t in range(2):
                        nc.sync.dma_start(dstt[t:32:2, :],
                                          seed[t0 + t:C_IN:8, :][0:16, :])
                    nc.sync.dma_start(dstt[32:64, :], dstt[0:32, :])
                    nc.sync.dma_start(dstt[64:128, :], dstt[0:64, :])
                    BC[tag] = dstt
                return dict(xs2=xs2, dt2=dt2, dAs=dAs, BC=BC, seed_t=seed)

            def scan(k, P):
                rev = k >= 2
                u2 = scn.tile([C_IN, L], fp16, tag="u2", name="u2", bufs=1)
                nc.vector.tensor_mul(u2[:], P["dt2"][:], P["xs2"][:])
                if DEBUG and k == 0 and P.get("dump0"):
                    P["dump0"]("u2k0", u2[:])
                hCs = []
                for gl in range(2):
                    Bt = P["BC"]["B0" if gl == 0 else "B1"]
                    Ct = P["BC"]["C0" if gl == 0 else "C1"]
                    bB = scn.tile([C_IN, L], fp16, tag="bB", name="bB",
                                  bufs=2)
                    nc.vector.tensor_mul(bB[:], u2[:], Bt[:])
                    h_c = scn.tile([C_IN, L], fp16, tag="h", name="h_c",
                                   bufs=2)
                    if not rev:
                        nc.vector.tensor_tensor_scan(
                            h_c[:], P["dAs"][gl][:], bB[:], 0.0,
                            OP.mult, OP.add)
                    else:
                        nc.vector.tensor_tensor_scan(
                            h_c[:][:, ::-1], P["dAs"][gl][:][:, ::-1],
                            bB[:][:, ::-1], 0.0, OP.mult, OP.add)
                    hC = scn.tile([C_IN, L], fp16, tag="hC", name="hC",
                                  bufs=2)
                    if DEBUG and k == 0 and gl == 0 and P.get("dump0"):
                        P["dump0"]("h0k0", h_c[:])
                    nc.vector.tensor_mul(hC[:], h_c[:], Ct[:])
                    hCs.append(hC)
                # ys reduction
                if k in (0, 2):        # rm pair -> y_rm (row-major)
                    for o in range(0, L, 512):
                        sw = min(512, L - o)
                        ps_ys = ps.tile([D, 512], f32, tag="ys", name="ps_ys",
                                        bufs=2)
                        if k == 0:
                            nc.tensor.matmul(ps_ys[:, :sw], lhsT_ds[:],
                                             xs_il[:, o:o + sw],
                                             start=True, stop=False,
                                             skip_group_check=True)
                        for gl in range(2):
                            nc.tensor.matmul(ps_ys[:, :sw], lhsT_ys[:],
                                             hCs[gl][:, o:o + sw],
                                             start=(gl == 0 and k != 0),
                                             stop=(gl == 1),
                                             skip_group_check=True)
                        if k == 0:
                            nc.scalar.copy(y_rm[:, o:o + sw], ps_ys[:, :sw])
                        else:
                            nc.vector.tensor_add(y_rm[:, o:o + sw],
                                                 y_rm[:, o:o + sw],
                                                 ps_ys[:, :sw])
                else:                  # cm pair -> y_cmg (h-major direct)
                    for j in range(5):
                        o = 480 * j
                        sw = min(480, L - o)
                        nw = sw // 48
                        ps_ys = ps.tile([D, 512], f32, tag="ys", name="ps_ys",
                                        bufs=2)
                        for gl in range(2):
                            nc.tensor.matmul(ps_ys[:, :sw], lhsT_ys[:],
                                             hCs[gl][:, o:o + sw],
                                             start=(gl == 0), stop=(gl == 1),
                                             skip_group_check=True)
                        # y_cm chunk o covers w-cols [10j, 10j+nw): write
                        # transposed into y_cmg (h-major)
                        dstv = y_cmg[:].rearrange(
                            "c (h w) -> c h w", h=48, w=48)[:, :, 10 * j:
                                                            10 * j + nw]
                        srcv = ps_ys[:, :sw].rearrange(
                            "c (w h) -> c h w", w=nw, h=48)
                        if k == 1:
                            nc.scalar.copy(dstv, srcv)
                        else:
                            nc.vector.tensor_add(dstv, dstv, srcv)

            P0 = prep(0)
            if DEBUG and i == 0:
                def dump(nm, ap):
                    nc.sync.dma_start(dbg_d[nm], ap)
                dump("xs_il", xs_il[:])
                dump("xs_cm", xs_cm[:])
                dump("sz", sz[:])
                dump("seed0", P0["seed_t"][:])
                dump("B0k0", P0["BC"]["B0"][:])
                dump("C1k0", P0["BC"]["C1"][:])
                dump("dt2k0", P0["dt2"][:])
                dump("dA0k0", P0["dAs"][0][:])
                dump("srctr", sr3[32:64, :])
                dump("mrctr", sr3[96:97, :])
                P0["dump0"] = dump
            P2 = prep(2)
            scan(0, P0)
            P1 = prep(1)
            scan(2, P2)

            # ---- rm ReduceScatter (issued mid-head) ----
            ri = dram.tile([4 * D, SHARD], fp16, tag="rs_rm_in", name="ri",
                           bufs=2)
            ro = dram.tile([D, SHARD], fp16, tag="rs_rm_out", name="ro",
                           bufs=2)
            nc.sync.dma_start(
                ri[:].rearrange("(r c) s -> c r s", r=4, c=D),
                y_rm[:].rearrange("c (r s) -> c r s", r=4, s=SHARD))
            nc.gpsimd.collective_compute(
                "ReduceScatter", OP.add, replica_groups=RG,
                ins=[ri[:].opt()], outs=[ro[:].opt()])

            P3 = prep(3)
            scan(1, P1)
            scan(3, P3)
            if DEBUG and i == 0:
                dump("y_rm", y_rm[:])
                dump("y_cmg", y_cmg[:])

            # ---- cm ReduceScatter (y_cmg + sz piggyback) ----
            ci_cm = dram.tile([4 * C_IN, SHARD], fp16, tag="rs_cm_in",
                              name="ci_cm", bufs=2)
            co_cm = dram.tile([C_IN, SHARD], fp16, tag="rs_cm_out",
                              name="co_cm", bufs=2)
            civ = ci_cm[:].rearrange("(r c) s -> c r s", r=4, c=C_IN)
            nc.sync.dma_start(
                civ[0:D], y_cmg[:].rearrange("c (r s) -> c r s", r=4,
                                             s=SHARD))
            nc.sync.dma_start(
                civ[D:C_IN], sz[:].rearrange("c (r s) -> c r s", r=4,
                                             s=SHARD))
            nc.gpsimd.collective_compute(
                "ReduceScatter", OP.add, replica_groups=RG,
                ins=[ci_cm[:].opt()], outs=[co_cm[:].opt()])

            # ---- sharded tail ----
            cm_sh = sml.tile([D, SHARD], fp16, tag="cm_sh", name="cm_sh")
            nc.sync.dma_start(cm_sh[:], co_cm[0:D, :])
            sz_sh = sml.tile([D, SHARD], fp16, tag="sz_sh", name="sz_sh")
            nc.sync.dma_start(sz_sh[:], co_cm[D:C_IN, :])
            rm_sh = sml.tile([D, SHARD], fp16, tag="rm_sh", name="rm_sh")
            nc.sync.dma_start(rm_sh[:], ro[:])
            if i == 0:
                s_sh = x_my_r[:, 0:SHARD]
            else:
                s_sh_t = sml.tile([C_H, SHARD], f32, tag="s_sh", name="s_sh")
                nc.vector.tensor_add(s_sh_t[:], prev_sh_ap,
                                     x_my_r[:, i * SHARD:(i + 1) * SHARD])
                s_sh = s_sh_t[:]
            y_sh = sml.tile([D, SHARD], fp16, tag="y_sh", name="y_sh")
            nc.vector.tensor_add(y_sh[:], rm_sh[:], cm_sh[:])
            ysq_t = sml.tile([D, SHARD], fp16, tag="ysqt", name="ysq_t")
            nc.scalar.square(ysq_t[:], y_sh[:])
            prev_sh = sml.tile([C_H, SHARD], f32, tag="prevsh",
                               name="prev_sh")
            for hh in range(2):
                o = hh * HCH
                ps_me2 = ps.tile([33, 512], f32, tag="st", name="ps_me2",
                                 bufs=1)
                nc.tensor.matmul(ps_me2[0:1, :HCH], lhsT_y1[:],
                                 y_sh[:, o:o + HCH], start=True, stop=True)
                nc.tensor.matmul(ps_me2[32:33, :HCH], lhsT_y1[:],
                                 ysq_t[:, o:o + HCH], start=True, stop=True)
                r_c = ln_smalls(ps_me2, HCH, "sm")
                m_c = sml.tile([1, HCH], f32, tag="mct", name="m_c")
                nc.scalar.copy(m_c[:], ps_me2[0:1, :HCH])
                ps_mb = ps.tile([D, 512], f32, tag="ys", name="ps_mb",
                                bufs=2)
                nc.tensor.matmul(ps_mb[:, :HCH], lhsT_bc[0:1, 0:D],
                                 m_c[:], start=True, stop=True)
                ps_rb = ps.tile([C_IN, 512], f32, tag="sp", name="ps_rb",
                                bufs=2)
                nc.tensor.matmul(ps_rb[0:C_H, :HCH], lhsT_bc[0:1, 0:C_H],
                                 r_c[:, :HCH], start=True, stop=True)
                ym = sml.tile([D, HCH], f32, tag="ym", name="ym")
                nc.vector.tensor_sub(ym[:], y_sh[:, o:o + HCH],
                                     ps_mb[:, :HCH])
                ysz = sml.tile([D, HCH], fp16, tag="ysz", name="ysz")
                nc.vector.tensor_mul(ysz[:], ym[:], sz_sh[:, o:o + HCH])
                ps_op = ps.tile([C_IN, 512], f32, tag="sp", name="ps_op",
                                bufs=2)
                nc.tensor.matmul(ps_op[0:C_H, :HCH], lhsT_op[:], ysz[:],
                                 start=True, stop=True)
                op_sb = sml.tile([C_H, HCH], f32, tag="op_sb", name="op_sb")
                nc.scalar.copy(op_sb[:], ps_op[0:C_H, :HCH])
                t_c = sml.tile([C_H, HCH], f32, tag="t_c", name="t_c")
                nc.vector.tensor_mul(t_c[:], op_sb[:], ps_rb[0:C_H, :HCH])
                nc.vector.scalar_tensor_tensor(
                    prev_sh[:, o:o + HCH],
                    s_sh[:, o:o + HCH], 1.0 + vs, t_c[:], OP.mult, OP.add)
            nc.sync.dma_start(outs_sh[32 * i:32 * (i + 1), :], prev_sh[:])
            prev_sh_ap = prev_sh[:]

            if i < HEAD - 1:
                prev16 = sml.tile([C_H, SHARD], fp16, tag="prev16",
                                  name="prev16")
                nc.scalar.copy(prev16[:], prev_sh[:])
                agi = dram.tile([C_H, SHARD], fp16, tag="ag_in", name="agi",
                                bufs=2)
                ago = dram.tile([4 * C_H, SHARD], fp16, tag="ag_out",
                                name="ago", bufs=2)
                nc.sync.dma_start(agi[:], prev16[:])
                nc.gpsimd.collective_compute(
                    "AllGather", OP.bypass, replica_groups=RG,
                    ins=[agi[:].opt()], outs=[ago[:].opt()])
                nc.sync.dma_start(
                    prev_full[:].rearrange("c (r s) -> c r s", r=4, s=SHARD),
                    ago[:].rearrange("(r c) s -> c r s", r=4, c=C_H))

        # ---- final: x_res = cvm*x_my + outs_sh (shard); LN over 128 ch ----
        xres = stg.tile([C_IN, SHARD], f32, tag="xres", name="xres")
        nc.vector.scalar_tensor_tensor(xres[:], x_my[:], cvm,
                                       outs_sh[:], OP.mult, OP.add)
        x16 = stg.tile([C_IN, SHARD], fp16, tag="x16", name="x16")
        nc.scalar.copy(x16[:], xres[:])
        xsq = stg.tile([C_IN, SHARD], fp16, tag="xsq", name="xsq")
        nc.scalar.square(xsq[:], xres[:])
        out_sb = stg.tile([C_IN, SHARD], f32, tag="out_sb", name="out_sb")
        for hh in range(2):
            o = hh * HCH
            ps_me3 = ps.tile([33, 512], f32, tag="st", name="ps_me3",
                             bufs=1)
            nc.tensor.matmul(ps_me3[0:1, :HCH], lhsT_stf[:, 0:1],
                             x16[:, o:o + HCH], start=True, stop=True)
            nc.tensor.matmul(ps_me3[32:33, :HCH], lhsT_stf[:, 1:2],
                             xsq[:, o:o + HCH], start=True, stop=True)
            r_c = ln_smalls(ps_me3, HCH, "sm")
            mr_c = sml.tile([1, HCH], f32, tag="smv", name="mr_c3")
            nc.vector.tensor_mul(mr_c[:], ps_me3[0:1, :HCH], r_c[:, :HCH])
            ps_ra = ps.tile([C_IN, 512], f32, tag="sp", name="ps_ra3",
                            bufs=2)
            nc.tensor.matmul(ps_ra[:, :HCH], lhsT_bc[:], r_c[:, :HCH],
                             start=True, stop=True)
            ps_rb = ps.tile([C_IN, 512], f32, tag="sp", name="ps_rb3",
                            bufs=2)
            nc.tensor.matmul(ps_rb[:, :HCH], lhsT_bc[:], mr_c[:],
                             start=True, stop=True)
            nc.vector.tensor_mul(out_sb[:, o:o + HCH], xres[:, o:o + HCH],
                                 ps_ra[:, :HCH])
            nc.vector.tensor_sub(out_sb[:, o:o + HCH], out_sb[:, o:o + HCH],
                                 ps_rb[:, :HCH])
            nc.vector.tensor_scalar(out_sb[:, o:o + HCH],
                                    out_sb[:, o:o + HCH],
                                    gamma[:], beta[:], OP.mult, OP.add)
        nc.sync.dma_start(out_d[:], out_sb[:])

    nc.compile()
    return nc


def _host_prep(inputs):
    """Build per-core input maps from full inputs."""
    fp16 = np.float16
    x = np.asarray(inputs["x"], np.float32)
    ln1_w = np.asarray(inputs["ln1_w"], np.float32)
    ln1_b = np.asarray(inputs["ln1_b"], np.float32)
    in_proj_w = np.asarray(inputs["in_proj_w"], np.float32)
    conv_w = np.asarray(inputs["conv_w"], np.float32)
    conv_b = np.asarray(inputs["conv_b"], np.float32)
    x_proj_w = np.asarray(inputs["x_proj_w"], np.float32)
    dt_proj_w = np.asarray(inputs["dt_proj_w"], np.float32)
    dt_proj_b = np.asarray(inputs["dt_proj_b"], np.float32)
    A_logs = np.asarray(inputs["A_logs"], np.float32)
    Ds = np.asarray(inputs["Ds"], np.float32)
    out_norm_w = np.asarray(inputs["out_norm_w"], np.float32)
    out_norm_b = np.asarray(inputs["out_norm_b"], np.float32)
    out_proj_w = np.asarray(inputs["out_proj_w"], np.float32)
    final_ln_w = np.asarray(inputs["final_ln_w"], np.float32)
    final_ln_b = np.asarray(inputs["final_ln_b"], np.float32)
    assert not np.any(out_norm_b), "out_norm_b must be zero (folded)"

    Wxx = (in_proj_w[0:D] * ln1_w[None, :]).astype(np.float32)    # (64, 32)
    Wz = (in_proj_w[D:C_IN] * ln1_w[None, :]).astype(np.float32)  # (64, 32)
    bias_ip = (in_proj_w @ ln1_b).astype(np.float32)              # (128,)
    w9 = np.ascontiguousarray(
        conv_w[:, :, 0, :].transpose(2, 0, 1).reshape(D, 9))      # (64, 9)
    colsum_xx = Wxx.sum(1)
    colsum_z = Wz.sum(1)

    # conv lhsT with duplicated-interleaved output: out col p = 2d+half
    lhsT_cv = np.zeros((99, 3, C_IN), fp16)
    for dy in range(3):
        for dx in range(3):
            tap = 3 * dy + dx
            blk = (w9[:, tap][None, :] * Wxx.T)          # (32c, 64d)
            cor = (-w9[:, tap] * colsum_xx)              # (64d,)
            mrrow = 96 if dx == 1 else (97 if dx == 0 else 98)
            for half in range(2):
                lhsT_cv[32 * dx:32 * dx + 32, dy, half::2] = blk.astype(fp16)
                lhsT_cv[mrrow, dy, half::2] = cor.astype(fp16)
    lhsT_z = np.zeros((99, D), fp16)
    lhsT_z[32:64, :] = Wz.T.astype(fp16)
    lhsT_z[96, :] = (-colsum_z).astype(fp16)

    bias_cv = np.zeros((C_IN, 1), np.float32)
    bcv = conv_b + w9.sum(1) * bias_ip[0:D]
    bias_cv[0::2, 0] = bcv
    bias_cv[1::2, 0] = bcv
    bias_z = bias_ip[D:C_IN].reshape(D, 1).astype(np.float32)

    lhsT_sq16 = np.full((C_H, 1), 1.0 / C_H, fp16)
    lhsT_y1 = np.full((D, 1), 1.0 / D, fp16)
    lhsT_bc = np.ones((1, C_IN), np.float32)

    # dt projection: contraction over il rows (2c+half -> /2), out il cols
    M = np.einsum("kdr,krc->kdc", dt_proj_w, x_proj_w[:, :DT_RANK, :])
    lhsT_M2 = np.zeros((C_IN, K, C_IN), fp16)
    dtb2 = np.zeros((C_IN, K), np.float32)
    for k in range(K):
        mh = (M[k].T / 2.0).astype(fp16)
        for half_in in range(2):
            for half_out in range(2):
                lhsT_M2[half_in::2, k, half_out::2] = mh
        dtb2[0::2, k] = dt_proj_b[k, :]
        dtb2[1::2, k] = dt_proj_b[k, :]

    A = -np.exp(A_logs)                                  # (K, 64, 16)
    Ds_q = (Ds.sum(0) / 4.0).astype(np.float32)          # (64,)
    # out_proj lhsT; includes out_norm gamma fold and the /4 for the sz RS
    W_op = (out_proj_w * out_norm_w[None, :]) / 4.0
    lhsT_op = np.ascontiguousarray(W_op.T).astype(fp16)  # (64, 32)

    lhsT_ys = np.zeros((C_IN, D), fp16)
    lhsT_ds = np.zeros((C_IN, D), fp16)
    for d in range(D):
        lhsT_ys[2 * d, d] = 1.0
        lhsT_ys[2 * d + 1, d] = 1.0
        lhsT_ds[2 * d, d] = Ds_q[d] / 2.0
        lhsT_ds[2 * d + 1, d] = Ds_q[d] / 2.0
    lhsT_stf = np.zeros((C_IN, 2), fp16)
    lhsT_stf[:, 0] = 1.0 / C_IN
    lhsT_stf[:, 1] = 1.0 / C_IN

    common = {
        "lhsT_cv": lhsT_cv, "lhsT_z": lhsT_z,
        "bias_cv": bias_cv, "bias_z": bias_z,
        "lhsT_sq16": lhsT_sq16,
        "lhsT_y1": lhsT_y1, "lhsT_bc": lhsT_bc,
        "lhsT_M2": lhsT_M2, "dtb2": dtb2,
        "lhsT_ys": lhsT_ys, "lhsT_ds": lhsT_ds,
        "lhsT_op": lhsT_op,
        "lhsT_stf": lhsT_stf,
        "gamma": final_ln_w.reshape(C_IN, 1),
        "beta": final_ln_b.reshape(C_IN, 1),
    }

    g = HEAD
    cg = C_IN // HEAD
    per_b = []
    per_b32 = []
    for b in range(B):
        xs = x[b].reshape(H, W, g, cg).transpose(0, 1, 3, 2).reshape(L, C_IN)
        cf = np.ascontiguousarray(xs.T)
        per_b32.append(cf)
        per_b.append(cf.astype(fp16))  # (128, L)

    in_maps = []
    for c in range(NCORES):
        b, nh = c // 4, c % 4
        A2 = np.zeros((C_IN, K, 2), np.float32)
        lhsT_seed = np.zeros((C_IN, K, C_IN), fp16)
        for k in range(K):
            for gl in range(2):
                for half in range(2):
                    n = 4 * nh + 2 * gl + half
                    for d in range(D):
                        A2[2 * d + half, k, gl] = A[k, d, n]
            # seed pattern: out col q = 8*j + t; t: 0,1=B gl0 h0,h1;
            # 2,3=B gl1; 4,5=C gl0; 6,7=C gl1
            for t in range(8):
                is_c = t // 4
                gl = (t % 4) // 2
                half = t % 2
                n = 4 * nh + 2 * gl + half
                row = x_proj_w[k, DT_RANK + 16 * is_c + n, :] / 2.0
                for half_in in range(2):
                    lhsT_seed[half_in::2, k, t::8] = \
                        row.astype(fp16)[:, None]
        x_shuf_cf = per_b[b]
        x_my = np.ascontiguousarray(
            per_b32[b][:, nh * SHARD:(nh + 1) * SHARD])
        x_my_r = np.zeros((C_H, 4 * SHARD), fp16)
        for i in range(HEAD):
            x_my_r[:, i * SHARD:(i + 1) * SHARD] = \
                x_my[32 * i:32 * (i + 1)].astype(fp16)
        in_maps.append(dict(common, x_shuf=x_shuf_cf, x_my=x_my,
                            x_my_r=x_my_r, A2=A2, lhsT_seed=lhsT_seed))
    vs = float(np.asarray(inputs["vss_skip"]).ravel()[0])
    cvm = float(np.asarray(inputs["cvm_skip"]).ravel()[0])
    return in_maps, vs, cvm


def kernel(**inputs) -> np.ndarray:
    from concourse.bass_utils import run_bass_kernel_spmd

    in_maps, vs, cvm = _host_prep(inputs)
    key = (vs, cvm)
    if key not in _cache:
        _cache[key] = _build(vs, cvm)
    nc = _cache[key]
    res = run_bass_kernel_spmd(nc, in_maps, core_ids=list(range(NCORES)))
    out = np.zeros((B, H, W, C_IN), np.float32)
    for b in range(B):
        full = np.zeros((C_IN, L), np.float32)
        for r in range(4):
            full[:, r * SHARD:(r + 1) * SHARD] = \
                res.results[4 * b + r]["out_cf"]
        out[b] = full.T.reshape(H, W, C_IN)
    return out


# revision 20
# speedup vs baseline: 1.2904x; 1.0047x over previous
"""Trainium2 Bass kernel for nn_CascadedVMambaBlock (v2).

Sharding: 8 cores; core c = (b, nh) with b = c//4, nh = c%4.
Each core processes sample b with state-dim slice n in [4nh, 4nh+4)
for ALL 4 scan directions k. Per-head combine is a ReduceScatter over
the 4-core b-group (each rank keeps an L-shard of 576 pixels), the
out_norm/out_proj tail runs sharded, and an AllGather rebuilds the
full next-stage input.

Layouts: channels-first (channels on partitions, L = 2304 free).
Scan lanes are INTERLEAVED: partition p = 2*d + half covers channel
d = p//2, state n = 4*nh + 2*gl + half.

v2 structure (vs v1):
- B/C row-expansion: the x_proj matmul emits a [128,512] seed whose
  output rows repeat the 8-row (B0h0,B0h1,B1h0,B1h1,C0..C1..) pattern
  16x (PE cost is free-dim only), one scalar copy -> fp16 seed tile,
  then per (gl,tensor) just 3 chained SBUF DMAs (32 rows gathered,
  then 2 log-doublings) instead of a 7-deep chain.
- Scan-prep (dt matmul, softplus, u2, dA, B/C) is computed full-L per
  direction k, 1-2 directions ahead, so the DVE runs the scan phase
  as one op per (k,gl) with no intra-k chaining.
- xs_cm is written directly from conv PSUM by a second strided silu;
  cm ys chunks write the h-major transpose (y_cmg) directly.
- KORD = (0,2,1,3): the rm ReduceScatter is issued mid-head and mostly
  hidden; only the cm ReduceScatter (y + sz piggyback) is exposed.
"""
import numpy as np

HEAD, C_IN, C_H = 4, 128, 32
D, N, K, DT_RANK = 64, 16, 4, 2
B, H, W = 2, 48, 48
L = H * W            # 2304
CS = 480             # LN1 chunk (10 image rows -> canvas-aligned)
LNCH = [(i * CS, min(CS, L - i * CS)) for i in range((L + CS - 1) // CS)]
SHARD = L // 4       # 576 pixels per rank
HCH = SHARD // 2     # 288-wide half-shard chunks
PADW = 50 * 51       # padded conv canvas (extra tail row for AP slack)
EPS = 1e-5
NCORES = 8
KORD = (0, 2, 1, 3)

_cache = {}


DEBUG = False


def _build(vs, cvm):
    import concourse.bass as bass
    import concourse.bacc as bacc
    import concourse.tile as tile
    import concourse.mybir as mybir
    from contextlib import ExitStack

    f32 = mybir.dt.float32
    fp16 = mybir.dt.float16
    AF = mybir.ActivationFunctionType
    OP = mybir.AluOpType

    import concourse.hw_specs as hw_specs
    _orig_gat = hw_specs.get_activation_tables
    _KEEP = {"natural_log_exp_and_others", "silu_and_others"}

    def _patched_gat(arch):
        t = _orig_gat(arch)
        return {k: (v if k in _KEEP else set()) for k, v in t.items()}

    bacc.get_activation_tables = _patched_gat

    nc = bacc.Bacc("TRN2", target_bir_lowering=False, debug=False,
                   enable_asserts=True, num_devices=NCORES)

    def din(name, shape, dtype=f32):
        return nc.dram_tensor(name, shape, dtype, kind="ExternalInput").ap()

    x_shuf_d = din("x_shuf", (C_IN, L), fp16)
    x_my_d = din("x_my", (C_IN, SHARD))
    lhsT_cv_d = din("lhsT_cv", (99, 3, C_IN), fp16)   # conv, il-duplicated out
    lhsT_z_d = din("lhsT_z", (99, D), fp16)           # z half (dy=1 only)
    bias_cv_d = din("bias_cv", (C_IN, 1))             # silu bias, il-duplicated
    bias_z_d = din("bias_z", (D, 1))
    x_myr_d = din("x_my_r", (C_H, 4 * SHARD), fp16)
    lhsT_sq16_d = din("lhsT_sq16", (C_H, 1), fp16)
    lhsT_y1_d = din("lhsT_y1", (D, 1), fp16)
    lhsT_bc_d = din("lhsT_bc", (1, C_IN))
    lhsT_M2_d = din("lhsT_M2", (C_IN, K, C_IN), fp16)
    dtb2_d = din("dtb2", (C_IN, K))
    A2_d = din("A2", (C_IN, K, 2))
    lhsT_seed_d = din("lhsT_seed", (C_IN, K, C_IN), fp16)
    lhsT_ys_d = din("lhsT_ys", (C_IN, D), fp16)
    lhsT_ds_d = din("lhsT_ds", (C_IN, D), fp16)       # Ds init (k=0 psum)
    lhsT_op_d = din("lhsT_op", (D, C_H), fp16)
    lhsT_stf_d = din("lhsT_stf", (C_IN, 2), fp16)
    gamma_d = din("gamma", (C_IN, 1))
    beta_d = din("beta", (C_IN, 1))

    out_d = nc.dram_tensor("out_cf", (C_IN, SHARD), f32,
                           kind="ExternalOutput").ap()
    dbg_d = {}
    if DEBUG:
        for nm, shp in (("xs_il", (C_IN, L)), ("xs_cm", (C_IN, L)),
                        ("sz", (D, L)), ("seed0", (C_IN, L)),
                        ("B0k0", (C_IN, L)), ("C1k0", (C_IN, L)),
                        ("dt2k0", (C_IN, L)), ("u2k0", (C_IN, L)),
                        ("dA0k0", (C_IN, L)), ("h0k0", (C_IN, L)),
                        ("y_rm", (D, L)), ("y_cmg", (D, L)),
                        ("srctr", (C_H, PADW)), ("mrctr", (1, PADW))):
            dbg_d[nm] = nc.dram_tensor("dbg_" + nm, shp, fp16,
                                       kind="ExternalOutput").ap()

    RG = [[0, 1, 2, 3], [4, 5, 6, 7]]

    with tile.TileContext(nc) as tc, ExitStack() as ctx:
        w_pool = ctx.enter_context(tc.tile_pool(name="weights", bufs=1))
        big = ctx.enter_context(tc.tile_pool(name="big", bufs=1))
        stg = ctx.enter_context(tc.tile_pool(name="stg", bufs=1))
        sml = ctx.enter_context(tc.tile_pool(name="sml", bufs=2))
        scn = ctx.enter_context(tc.tile_pool(name="scn", bufs=2))
        ps = ctx.enter_context(tc.tile_pool(name="ps", bufs=1, space="PSUM"))
        dram = ctx.enter_context(tc.tile_pool(name="dram", bufs=2, space="DRAM"))

        def wload(ap_d, shape, dtype=f32):
            t = w_pool.tile(list(shape), dtype, name=ap_d.tensor.name + "_sb")
            src = ap_d if ap_d.dtype == dtype else ap_d.bitcast(dtype)
            nc.sync.dma_start(t[:], src)
            return t

        x_my = wload(x_my_d, (C_IN, SHARD))
        lhsT_cv = wload(lhsT_cv_d, (99, 3, C_IN), fp16)
        lhsT_z = wload(lhsT_z_d, (99, D), fp16)
        bias_cv = wload(bias_cv_d, (C_IN, 1))
        bias_z = wload(bias_z_d, (D, 1))
        x_my_r = wload(x_myr_d, (C_H, 4 * SHARD), fp16)
        lhsT_sq16 = wload(lhsT_sq16_d, (C_H, 1), fp16)
        lhsT_y1 = wload(lhsT_y1_d, (D, 1), fp16)
        lhsT_bc = wload(lhsT_bc_d, (1, C_IN))
        lhsT_M2 = wload(lhsT_M2_d, (C_IN, K, C_IN), fp16)
        dtb2 = wload(dtb2_d, (C_IN, K))
        A2 = wload(A2_d, (C_IN, K, 2))
        lhsT_seed = wload(lhsT_seed_d, (C_IN, K, C_IN), fp16)
        lhsT_ys = wload(lhsT_ys_d, (C_IN, D), fp16)
        lhsT_ds = wload(lhsT_ds_d, (C_IN, D), fp16)
        lhsT_op = wload(lhsT_op_d, (D, C_H), fp16)
        lhsT_stf = wload(lhsT_stf_d, (C_IN, 2), fp16)
        gamma = wload(gamma_d, (C_IN, 1))
        beta = wload(beta_d, (C_IN, 1))

        # persistent tiles
        # canvas rows: 0:32 sr(dx0), 32:64 sr(center/dx1), 64:96 sr(dx2),
        #              96 mr(center/dx1), 97 mr(dx0), 98 mr(dx2)
        # (mr center must sit on a quarter-aligned partition for DVE writes)
        sr3 = big.tile([99, PADW], fp16)
        nc.vector.memset(sr3[:], 0.0)
        xs_il = big.tile([C_IN, L], fp16)    # duplicated-interleaved xs (rm)
        xs_cm = big.tile([C_IN, L], fp16)    # col-major layout (from conv)
        outs_sh = big.tile([C_IN, SHARD], f32)
        prev_full = big.tile([C_H, L], fp16)
        y_rm = big.tile([D, L], fp16)
        y_cmg = big.tile([D, L], fp16)       # cm result, already h-major
        sz = big.tile([D, L], fp16)          # silu(z), full L

        # canvas views
        CTR = sr3[32:64, :]

        def cview(rows, j, nr):
            # canvas view writing sr[h, w] at position 51 + 50h + w: the
            # CENTER region is left-shifted by one col (R_1[h, b] = sr[h, b+1])
            base = 51 + 500 * j
            return rows[:, base:base + 50 * nr].rearrange(
                "c (h w) -> c h w", h=nr, w=50)[:, :, 0:48]

        def ln_smalls(ps_me, w, tagp):
            # ps_me psum: row 0 = mean, row 32 = E[x^2]
            m2_c = sml.tile([1, CS], f32, tag=tagp + "m2", name="m2_c")
            nc.scalar.square(m2_c[:, :w], ps_me[0:1, :w])
            var_c = sml.tile([1, CS], f32, tag=tagp + "v", name="var_c")
            nc.vector.scalar_tensor_tensor(var_c[:, :w], ps_me[32:33, :w],
                                           EPS, m2_c[:, :w], OP.add,
                                           OP.subtract)
            lnv_c = sml.tile([1, CS], f32, tag=tagp + "m2", name="lnv_c")
            nc.scalar.activation(lnv_c[:, :w], var_c[:, :w], AF.Ln)
            r_c = sml.tile([1, CS], f32, tag=tagp + "r", name="r_c")
            nc.scalar.activation(r_c[:, :w], lnv_c[:, :w], AF.Exp, scale=-0.5)
            return r_c

        s_t = None
        prev_sh_ap = None
        for i in range(HEAD):
            # ---- stage input s (32, L) ----
            chunk_sb = sml.tile([C_H, L], fp16, tag="s_cs", name="chunk_sb",
                                bufs=1)
            nc.sync.dma_start(chunk_sb[:], x_shuf_d[32 * i:32 * (i + 1), :])
            if i == 0:
                s_t = chunk_sb[:]
            else:
                nc.vector.tensor_add(prev_full[:], prev_full[:], chunk_sb[:])
                s_t = prev_full[:]

            # ---- LN1: stats + sr/mr written into canvas center ----
            for ci, (o, w) in enumerate(LNCH):
                nr = w // 48
                sq_c = sml.tile([C_H, CS], fp16, tag="sqc", name="sq_c")
                nc.scalar.square(sq_c[:, :w], s_t[:, o:o + w])
                ps_me = ps.tile([33, 512], f32, tag="st", name="ps_me",
                                bufs=1)
                nc.tensor.matmul(ps_me[0:1, :w], lhsT_sq16[:],
                                 s_t[:, o:o + w], start=True, stop=True)
                nc.tensor.matmul(ps_me[32:33, :w], lhsT_sq16[:],
                                 sq_c[:, :w], start=True, stop=True)
                r_c = ln_smalls(ps_me, w, "sm")
                # mr -> canvas row 96 (center), strided over rows
                nc.vector.tensor_mul(
                    cview(sr3[96:97, :], ci, nr), ps_me[0:1, :w].rearrange(
                        "c (h w) -> c h w", h=nr, w=48),
                    r_c[:, :w].rearrange("c (h w) -> c h w", h=nr, w=48))
                ps_rr = ps.tile([C_IN, 512], f32, tag="sp", name="ps_rr",
                                bufs=2)
                nc.tensor.matmul(ps_rr[0:C_H, :w], lhsT_bc[0:1, 0:C_H],
                                 r_c[:, :w], start=True, stop=True)
                nc.vector.tensor_mul(
                    cview(CTR, ci, nr),
                    s_t[:, o:o + w].rearrange("c (h w) -> c h w", h=nr, w=48),
                    ps_rr[0:C_H, :w].rearrange("c (h w) -> c h w", h=nr, w=48))

            # ---- dx-shifted copies (center -> dx0/dx2 regions) ----
            ctr_flat = sr3[32:64, 51:51 + 2400].rearrange(
                "c (h w) -> c h w", h=48, w=50)
            mr_flat = sr3[96:97, 51:51 + 2400].rearrange(
                "c (h w) -> c h w", h=48, w=50)
            for dx in (0, 2):
                b0 = max(0, dx - 1)
                b1 = min(47, 46 + dx)          # inclusive src col range
                wdt = b1 - b0 + 1
                base = 52 + b0 - dx
                mrrow = 97 if dx == 0 else 98
                dv = sr3[32 * dx:32 * dx + 32, base:base + 2400].rearrange(
                    "c (h w) -> c h w", h=48, w=50)[:, :, 0:wdt]
                nc.sync.dma_start(dv, ctr_flat[:, :, b0:b1 + 1])
                mv = sr3[mrrow:mrrow + 1, base:base + 2400].rearrange(
                    "c (h w) -> c h w", h=48, w=50)[:, :, 0:wdt]
                nc.sync.dma_start(mv, mr_flat[:, :, b0:b1 + 1])

            # ---- conv/in_proj: accumulated matmuls; dual silu out ----
            h0 = 0
            while h0 < 48:
                nr = min(10, 48 - h0)
                wch = nr * 48
                ps_cv = ps.tile([C_IN, 512], f32, tag="cv", name="ps_cv",
                                bufs=1)
                pv = ps_cv[:, :wch].rearrange("c (h w) -> c h w", h=nr, w=48)
                ps_z = ps.tile([D, 512], f32, tag="ys", name="ps_z", bufs=2)
                zv = ps_z[:, :wch].rearrange("c (h w) -> c h w", h=nr, w=48)
                for dy in range(3):
                    base = 1 + 50 * dy + 50 * h0
                    rv = sr3[:, base:base + 50 * nr].rearrange(
                        "c (h w) -> c h w", h=nr, w=50)[:, :, 0:48]
                    nc.tensor.matmul(pv, lhsT_cv[:, dy, :], rv,
                                     start=(dy == 0), stop=(dy == 2))
                    if dy == 1:
                        nc.tensor.matmul(zv, lhsT_z[:], rv,
                                         start=True, stop=True)
                nc.scalar.activation(sz[:, 48 * h0:48 * h0 + wch],
                                     ps_z[:, :wch], AF.Silu, bias=bias_z[:])
                nc.scalar.activation(xs_il[:, 48 * h0:48 * h0 + wch],
                                     ps_cv[:, :wch], AF.Silu, bias=bias_cv[:])
                # second silu: write col-major layout directly
                cm_dst = xs_cm[:].rearrange(
                    "c (w h) -> c w h", w=48, h=48)[:, :, h0:h0 + nr]
                nc.scalar.activation(
                    cm_dst, ps_cv[:, :wch].rearrange(
                        "c (h w) -> c w h", h=nr, w=48),
                    AF.Silu, bias=bias_cv[:])
                h0 += nr

            # ---- scan preps + scans ----
            def prep(k):
                xs2 = xs_il if k in (0, 2) else xs_cm
                ech = scn.tile([C_IN, L], fp16, tag="ech", name="ech", bufs=1)
                for o in range(0, L, 1024):
                    cw = min(1024, L - o)
                    ps_dt = ps.tile([C_IN, 1024], f32, tag="pdt",
                                    name="ps_dt", bufs=1)
                    for so in range(0, cw, 512):
                        sw = min(512, cw - so)
                        nc.tensor.matmul(ps_dt[:, so:so + sw],
                                         lhsT_M2[:, k, :],
                                         xs2[:, o + so:o + so + sw],
                                         start=True, stop=True)
                    nc.scalar.activation(ech[:, o:o + cw], ps_dt[:, :cw],
                                         AF.Exp, bias=dtb2[:, k:k + 1])
                dt2 = scn.tile([C_IN, L], fp16, tag="dt2", name="dt2", bufs=2)
                nc.scalar.activation(dt2[:], ech[:], AF.Ln, bias=1.0)
                dAs = []
                for gl in range(2):
                    dA = scn.tile([C_IN, L], fp16, tag=f"dA{gl}", name="dA",
                                  bufs=2)
                    nc.scalar.activation(dA[:], dt2[:], AF.Exp,
                                         scale=A2[:, k, gl:gl + 1])
                    dAs.append(dA)
                seed = scn.tile([C_IN, L], fp16, tag="seed", name="seed",
                                bufs=1)
                for o in range(0, L, 512):
                    sw = min(512, L - o)
                    ps_sd = ps.tile([C_IN, 512], f32, tag="sp", name="ps_sd",
                                    bufs=2)
                    nc.tensor.matmul(ps_sd[:, :sw], lhsT_seed[:, k, :],
                                     xs2[:, o:o + sw], start=True, stop=True)
                    nc.scalar.copy(seed[:, o:o + sw], ps_sd[:, :sw])
                # expansion: 4 DMAs per (tensor, gl); SBUF APs may only
                # stride partitions in dim 0, so gather each parity separately
                BC = {}
                for ti, tag in enumerate(("B0", "B1", "C0", "C1")):
                    t0 = (ti % 2) * 2 + (ti // 2) * 4
                    dstt = scn.tile([C_IN, L], fp16, tag=tag, name=tag,
                                    bufs=2)
                    for t in range(2):
                        nc.sync.dma_start(dstt[t:32:2, :],
                                          seed[t0 + t:C_IN:8, :][0:16, :])
                    nc.sync.dma_start(dstt[32:64, :], dstt[0:32, :])
                    nc.sync.dma_start(dstt[64:128, :], dstt[0:64, :])
                    BC[tag] = dstt
                return dict(xs2=xs2, dt2=dt2, dAs=dAs, BC=BC, seed_t=seed)

            def scan(k, P):
                rev = k >= 2
                u2 = scn.tile([C_IN, L], fp16, tag="u2", name="u2", bufs=1)
                nc.vector.tensor_mul(u2[:], P["dt2"][:], P["xs2"][:])
                hCs = []
                for gl in range(2):
                    Bt = P["BC"]["B0" if gl == 0 else "B1"]
                    Ct = P["BC"]["C0" if gl == 0 else "C1"]
                    bB = scn.tile([C_IN, L], fp16, tag="bB", name="bB",
                                  bufs=2)
                    nc.vector.tensor_mul(bB[:], u2[:], Bt[:])
                    h_c = scn.tile([C_IN, L], fp16, tag="h", name="h_c",
                                   bufs=2)
                    if not rev:
                        nc.vector.tensor_tensor_scan(
                            h_c[:], P["dAs"][gl][:], bB[:], 0.0,
                            OP.mult, OP.add)
                    else:
                        nc.vector.tensor_tensor_scan(
                            h_c[:][:, ::-1], P["dAs"][gl][:][:, ::-1],
                            bB[:][:, ::-1], 0.0, OP.mult, OP.add)
                    hC = scn.tile([C_IN, L], fp16, tag="hC", name="hC",
                                  bufs=2)
                    nc.vector.tensor_mul(hC[:], h_c[:], Ct[:])
                    hCs.append(hC)
                # ys reduction
                if k in (0, 2):        # rm pair -> y_rm (row-major)
                    for o in range(0, L, 512):
                        sw = min(512, L - o)
                        ps_ys = ps.tile([D, 512], f32, tag="ys", name="ps_ys",
                                        bufs=2)
                        if k == 0:
                            nc.tensor.matmul(ps_ys[:, :sw], lhsT_ds[:],
                                             xs_il[:, o:o + sw],
                                             start=True, stop=False,
                                             skip_group_check=True)
                        for gl in range(2):
                            nc.tensor.matmul(ps_ys[:, :sw], lhsT_ys[:],
                                             hCs[gl][:, o:o + sw],
                                             start=(gl == 0 and k != 0),
                                             stop=(gl == 1),
                                             skip_group_check=True)
                        if k == 0:
                            nc.scalar.copy(y_rm[:, o:o + sw], ps_ys[:, :sw])
                        else:
                            nc.vector.tensor_add(y_rm[:, o:o + sw],
                                                 y_rm[:, o:o + sw],
                                                 ps_ys[:, :sw])
                else:                  # cm pair -> y_cmg (h-major direct)
                    for j in range(5):
                        o = 480 * j
                        sw = min(480, L - o)
                        nw = sw // 48
                        ps_ys = ps.tile([D, 512], f32, tag="ys", name="ps_ys",
                                        bufs=2)
                        for gl in range(2):
                            nc.tensor.matmul(ps_ys[:, :sw], lhsT_ys[:],
                                             hCs[gl][:, o:o + sw],
                                             start=(gl == 0), stop=(gl == 1),
                                             skip_group_check=True)
                        # y_cm chunk o covers w-cols [10j, 10j+nw): write
                        # transposed into y_cmg (h-major)
                        dstv = y_cmg[:].rearrange(
                            "c (h w) -> c h w", h=48, w=48)[:, :, 10 * j:
                                                            10 * j + nw]
                        srcv = ps_ys[:, :sw].rearrange(
                            "c (w h) -> c h w", w=nw, h=48)
                        if k == 1:
                            nc.scalar.copy(dstv, srcv)
                        else:
                            nc.vector.tensor_add(dstv, dstv, srcv)

            P1 = prep(1)
            P3 = prep(3)
            scan(1, P1)
            P0 = prep(0)
            scan(3, P3)

            # ---- cm ReduceScatter (y_cmg + sz piggyback), issued mid-head
            ci_cm = dram.tile([4 * C_IN, SHARD], fp16, tag="rs_cm_in",
                              name="ci_cm", bufs=2)
            co_cm = dram.tile([C_IN, SHARD], fp16, tag="rs_cm_out",
                              name="co_cm", bufs=2)
            civ = ci_cm[:].rearrange("(r c) s -> c r s", r=4, c=C_IN)
            nc.sync.dma_start(
                civ[0:D], y_cmg[:].rearrange("c (r s) -> c r s", r=4,
                                             s=SHARD))
            nc.sync.dma_start(
                civ[D:C_IN], sz[:].rearrange("c (r s) -> c r s", r=4,
                                             s=SHARD))
            nc.gpsimd.collective_compute(
                "ReduceScatter", OP.add, replica_groups=RG,
                ins=[ci_cm[:].opt()], outs=[co_cm[:].opt()])

            P2 = prep(2)
            scan(0, P0)
            scan(2, P2)

            # ---- rm ReduceScatter (smaller payload; exposed at tail) ----
            ri = dram.tile([4 * D, SHARD], fp16, tag="rs_rm_in", name="ri",
                           bufs=2)
            ro = dram.tile([D, SHARD], fp16, tag="rs_rm_out", name="ro",
                           bufs=2)
            nc.sync.dma_start(
                ri[:].rearrange("(r c) s -> c r s", r=4, c=D),
                y_rm[:].rearrange("c (r s) -> c r s", r=4, s=SHARD))
            nc.gpsimd.collective_compute(
                "ReduceScatter", OP.add, replica_groups=RG,
                ins=[ri[:].opt()], outs=[ro[:].opt()])

            # ---- sharded tail ----
            cm_sh = sml.tile([D, SHARD], fp16, tag="cm_sh", name="cm_sh")
            nc.sync.dma_start(cm_sh[:], co_cm[0:D, :])
            sz_sh = sml.tile([D, SHARD], fp16, tag="sz_sh", name="sz_sh")
            nc.sync.dma_start(sz_sh[:], co_cm[D:C_IN, :])
            rm_sh = sml.tile([D, SHARD], fp16, tag="rm_sh", name="rm_sh")
            nc.sync.dma_start(rm_sh[:], ro[:])
            if i == 0:
                s_sh = x_my_r[:, 0:SHARD]
            else:
                s_sh_t = sml.tile([C_H, SHARD], f32, tag="s_sh", name="s_sh")
                nc.vector.tensor_add(s_sh_t[:], prev_sh_ap,
                                     x_my_r[:, i * SHARD:(i + 1) * SHARD])
                s_sh = s_sh_t[:]
            y_sh = sml.tile([D, SHARD], fp16, tag="y_sh", name="y_sh")
            nc.vector.tensor_add(y_sh[:], rm_sh[:], cm_sh[:])
            ysq_t = sml.tile([D, SHARD], fp16, tag="ysqt", name="ysq_t")
            nc.scalar.square(ysq_t[:], y_sh[:])
            prev_sh = sml.tile([C_H, SHARD], f32, tag="prevsh",
                               name="prev_sh")
            for hh in range(2):
                o = hh * HCH
                ps_me2 = ps.tile([33, 512], f32, tag="st", name="ps_me2",
                                 bufs=1)
                nc.tensor.matmul(ps_me2[0:1, :HCH], lhsT_y1[:],
                                 y_sh[:, o:o + HCH], start=True, stop=True)
                nc.tensor.matmul(ps_me2[32:33, :HCH], lhsT_y1[:],
                                 ysq_t[:, o:o + HCH], start=True, stop=True)
                r_c = ln_smalls(ps_me2, HCH, "sm")
                m_c = sml.tile([1, HCH], f32, tag="mct", name="m_c")
                nc.scalar.copy(m_c[:], ps_me2[0:1, :HCH])
                ps_mb = ps.tile([D, 512], f32, tag="ys", name="ps_mb",
                                bufs=2)
                nc.tensor.matmul(ps_mb[:, :HCH], lhsT_bc[0:1, 0:D],
                                 m_c[:], start=True, stop=True)
                ps_rb = ps.tile([C_IN, 512], f32, tag="sp", name="ps_rb",
                                bufs=2)
                nc.tensor.matmul(ps_rb[0:C_H, :HCH], lhsT_bc[0:1, 0:C_H],
                                 r_c[:, :HCH], start=True, stop=True)
                ym = sml.tile([D, HCH], f32, tag="ym", name="ym")
                nc.vector.tensor_sub(ym[:], y_sh[:, o:o + HCH],
                                     ps_mb[:, :HCH])
                ysz = sml.tile([D, HCH], fp16, tag="ysz", name="ysz")
                nc.vector.tensor_mul(ysz[:], ym[:], sz_sh[:, o:o + HCH])
                ps_op = ps.tile([C_IN, 512], f32, tag="sp", name="ps_op",
                                bufs=2)
                nc.tensor.matmul(ps_op[0:C_H, :HCH], lhsT_op[:], ysz[:],
                                 start=True, stop=True)
                op_sb = sml.tile([C_H, HCH], f32, tag="op_sb", name="op_sb")
                nc.scalar.copy(op_sb[:], ps_op[0:C_H, :HCH])
                t_c = sml.tile([C_H, HCH], f32, tag="t_c", name="t_c")
                nc.vector.tensor_mul(t_c[:], op_sb[:], ps_rb[0:C_H, :HCH])
                nc.vector.scalar_tensor_tensor(
                    prev_sh[:, o:o + HCH],
                    s_sh[:, o:o + HCH], 1.0 + vs, t_c[:], OP.mult, OP.add)
            nc.sync.dma_start(outs_sh[32 * i:32 * (i + 1), :], prev_sh[:])
            prev_sh_ap = prev_sh[:]

            if i < HEAD - 1:
                prev16 = sml.tile([C_H, SHARD], fp16, tag="prev16",
                                  name="prev16")
                nc.scalar.copy(prev16[:], prev_sh[:])
                agi = dram.tile([C_H, SHARD], fp16, tag="ag_in", name="agi",
                                bufs=2)
                ago = dram.tile([4 * C_H, SHARD], fp16, tag="ag_out",
                                name="ago", bufs=2)
                nc.sync.dma_start(agi[:], prev16[:])
                nc.gpsimd.collective_compute(
                    "AllGather", OP.bypass, replica_groups=RG,
                    ins=[agi[:].opt()], outs=[ago[:].opt()])
                nc.sync.dma_start(
                    prev_full[:].rearrange("c (r s) -> c r s", r=4, s=SHARD),
                    ago[:].rearrange("(r c) s -> c r s", r=4, c=C_H))

        # ---- final: x_res = cvm*x_my + outs_sh (shard); LN over 128 ch ----
        xres = stg.tile([C_IN, SHARD], f32, tag="xres", name="xres")
        nc.vector.scalar_tensor_tensor(xres[:], x_my[:], cvm,
                                       outs_sh[:], OP.mult, OP.add)
        x16 = stg.tile([C_IN, SHARD], fp16, tag="x16", name="x16")
        nc.scalar.copy(x16[:], xres[:])
        xsq = stg.tile([C_IN, SHARD], fp16, tag="xsq", name="xsq")
        nc.scalar.square(xsq[:], xres[:])
        out_sb = stg.tile([C_IN, SHARD], f32, tag="out_sb", name="out_sb")
        for hh in range(2):
            o = hh * HCH
            ps_me3 = ps.tile([33, 512], f32, tag="st", name="ps_me3",
                             bufs=1)
            nc.tensor.matmul(ps_me3[0:1, :HCH], lhsT_stf[:, 0:1],
                             x16[:, o:o + HCH], start=True, stop=True)
            nc.tensor.matmul(ps_me3[32:33, :HCH], lhsT_stf[:, 1:2],
                             xsq[:, o:o + HCH], start=True, stop=True)
            r_c = ln_smalls(ps_me3, HCH, "sm")
            mr_c = sml.tile([1, HCH], f32, tag="smv", name="mr_c3")
            nc.vector.tensor_mul(mr_c[:], ps_me3[0:1, :HCH], r_c[:, :HCH])
            ps_ra = ps.tile([C_IN, 512], f32, tag="sp", name="ps_ra3",
                            bufs=2)
            nc.tensor.matmul(ps_ra[:, :HCH], lhsT_bc[:], r_c[:, :HCH],
                             start=True, stop=True)
            ps_rb = ps.tile([C_IN, 512], f32, tag="sp", name="ps_rb3",
                            bufs=2)
            nc.tensor.matmul(ps_rb[:, :HCH], lhsT_bc[:], mr_c[:],
                             start=True, stop=True)
            nc.vector.tensor_mul(out_sb[:, o:o + HCH], xres[:, o:o + HCH],
                                 ps_ra[:, :HCH])
            nc.vector.tensor_sub(out_sb[:, o:o + HCH], out_sb[:, o:o + HCH],
                                 ps_rb[:, :HCH])
            nc.vector.tensor_scalar(out_sb[:, o:o + HCH],
                                    out_sb[:, o:o + HCH],
                                    gamma[:], beta[:], OP.mult, OP.add)
        nc.sync.dma_start(out_d[:], out_sb[:])

    nc.compile()
    return nc


def _host_prep(inputs):
    """Build per-core input maps from full inputs."""
    fp16 = np.float16
    x = np.asarray(inputs["x"], np.float32)
    ln1_w = np.asarray(inputs["ln1_w"], np.float32)
    ln1_b = np.asarray(inputs["ln1_b"], np.float32)
    in_proj_w = np.asarray(inputs["in_proj_w"], np.float32)
    conv_w = np.asarray(inputs["conv_w"], np.float32)
    conv_b = np.asarray(inputs["conv_b"], np.float32)
    x_proj_w = np.asarray(inputs["x_proj_w"], np.float32)
    dt_proj_w = np.asarray(inputs["dt_proj_w"], np.float32)
    dt_proj_b = np.asarray(inputs["dt_proj_b"], np.float32)
    A_logs = np.asarray(inputs["A_logs"], np.float32)
    Ds = np.asarray(inputs["Ds"], np.float32)
    out_norm_w = np.asarray(inputs["out_norm_w"], np.float32)
    out_norm_b = np.asarray(inputs["out_norm_b"], np.float32)
    out_proj_w = np.asarray(inputs["out_proj_w"], np.float32)
    final_ln_w = np.asarray(inputs["final_ln_w"], np.float32)
    final_ln_b = np.asarray(inputs["final_ln_b"], np.float32)
    assert not np.any(out_norm_b), "out_norm_b must be zero (folded)"

    Wxx = (in_proj_w[0:D] * ln1_w[None, :]).astype(np.float32)    # (64, 32)
    Wz = (in_proj_w[D:C_IN] * ln1_w[None, :]).astype(np.float32)  # (64, 32)
    bias_ip = (in_proj_w @ ln1_b).astype(np.float32)              # (128,)
    w9 = np.ascontiguousarray(
        conv_w[:, :, 0, :].transpose(2, 0, 1).reshape(D, 9))      # (64, 9)
    colsum_xx = Wxx.sum(1)
    colsum_z = Wz.sum(1)

    # conv lhsT with duplicated-interleaved output: out col p = 2d+half
    lhsT_cv = np.zeros((99, 3, C_IN), fp16)
    for dy in range(3):
        for dx in range(3):
            tap = 3 * dy + dx
            blk = (w9[:, tap][None, :] * Wxx.T)          # (32c, 64d)
            cor = (-w9[:, tap] * colsum_xx)              # (64d,)
            mrrow = 96 if dx == 1 else (97 if dx == 0 else 98)
            for half in range(2):
                lhsT_cv[32 * dx:32 * dx + 32, dy, half::2] = blk.astype(fp16)
                lhsT_cv[mrrow, dy, half::2] = cor.astype(fp16)
    lhsT_z = np.zeros((99, D), fp16)
    lhsT_z[32:64, :] = Wz.T.astype(fp16)
    lhsT_z[96, :] = (-colsum_z).astype(fp16)

    bias_cv = np.zeros((C_IN, 1), np.float32)
    bcv = conv_b + w9.sum(1) * bias_ip[0:D]
    bias_cv[0::2, 0] = bcv
    bias_cv[1::2, 0] = bcv
    bias_z = bias_ip[D:C_IN].reshape(D, 1).astype(np.float32)

    lhsT_sq16 = np.full((C_H, 1), 1.0 / C_H, fp16)
    lhsT_y1 = np.full((D, 1), 1.0 / D, fp16)
    lhsT_bc = np.ones((1, C_IN), np.float32)

    # dt projection: contraction over il rows (2c+half -> /2), out il cols
    M = np.einsum("kdr,krc->kdc", dt_proj_w, x_proj_w[:, :DT_RANK, :])
    lhsT_M2 = np.zeros((C_IN, K, C_IN), fp16)
    dtb2 = np.zeros((C_IN, K), np.float32)
    for k in range(K):
        mh = (M[k].T / 2.0).astype(fp16)
        for half_in in range(2):
            for half_out in range(2):
                lhsT_M2[half_in::2, k, half_out::2] = mh
        dtb2[0::2, k] = dt_proj_b[k, :]
        dtb2[1::2, k] = dt_proj_b[k, :]

    A = -np.exp(A_logs)                                  # (K, 64, 16)
    Ds_q = (Ds.sum(0) / 4.0).astype(np.float32)          # (64,)
    # out_proj lhsT; includes out_norm gamma fold and the /4 for the sz RS
    W_op = (out_proj_w * out_norm_w[None, :]) / 4.0
    lhsT_op = np.ascontiguousarray(W_op.T).astype(fp16)  # (64, 32)

    lhsT_ys = np.zeros((C_IN, D), fp16)
    lhsT_ds = np.zeros((C_IN, D), fp16)
    for d in range(D):
        lhsT_ys[2 * d, d] = 1.0
        lhsT_ys[2 * d + 1, d] = 1.0
        lhsT_ds[2 * d, d] = Ds_q[d] / 2.0
        lhsT_ds[2 * d + 1, d] = Ds_q[d] / 2.0
    lhsT_stf = np.zeros((C_IN, 2), fp16)
    lhsT_stf[:, 0] = 1.0 / C_IN
    lhsT_stf[:, 1] = 1.0 / C_IN

    common = {
        "lhsT_cv": lhsT_cv, "lhsT_z": lhsT_z,
        "bias_cv": bias_cv, "bias_z": bias_z,
        "lhsT_sq16": lhsT_sq16,
        "lhsT_y1": lhsT_y1, "lhsT_bc": lhsT_bc,
        "lhsT_M2": lhsT_M2, "dtb2": dtb2,
        "lhsT_ys": lhsT_ys, "lhsT_ds": lhsT_ds,
        "lhsT_op": lhsT_op,
        "lhsT_stf": lhsT_stf,
        "gamma": final_ln_w.reshape(C_IN, 1),
        "beta": final_ln_b.reshape(C_IN, 1),
    }

    g = HEAD
    cg = C_IN // HEAD
    per_b = []
    per_b32 = []
    for b in range(B):
        xs = x[b].reshape(H, W, g, cg).transpose(0, 1, 3, 2).reshape(L, C_IN)
        cf = np.ascontiguousarray(xs.T)
        per_b32.append(cf)
        per_b.append(cf.astype(fp16))  # (128, L)

    in_maps = []
    for c in range(NCORES):
        b, nh = c // 4, c % 4
        A2 = np.zeros((C_IN, K, 2), np.float32)
        lhsT_seed = np.zeros((C_IN, K, C_IN), fp16)
        for k in range(K):
            for gl in range(2):
                for half in range(2):
                    n = 4 * nh + 2 * gl + half
                    for d in range(D):
                        A2[2 * d + half, k, gl] = A[k, d, n]
            # seed pattern: out col q = 8*j + t; t: 0,1=B gl0 h0,h1;
            # 2,3=B gl1; 4,5=C gl0; 6,7=C gl1
            for t in range(8):
                is_c = t // 4
                gl = (t % 4) // 2
                half = t % 2
                n = 4 * nh + 2 * gl + half
                row = x_proj_w[k, DT_RANK + 16 * is_c + n, :] / 2.0
                for half_in in range(2):
                    lhsT_seed[half_in::2, k, t::8] = \
                        row.astype(fp16)[:, None]
        x_shuf_cf = per_b[b]
        x_my = np.ascontiguousarray(
            per_b32[b][:, nh * SHARD:(nh + 1) * SHARD])
        x_my_r = np.zeros((C_H, 4 * SHARD), fp16)
        for i in range(HEAD):
            x_my_r[:, i * SHARD:(i + 1) * SHARD] = \
                x_my[32 * i:32 * (i + 1)].astype(fp16)
        in_maps.append(dict(common, x_shuf=x_shuf_cf, x_my=x_my,
                            x_my_r=x_my_r, A2=A2, lhsT_seed=lhsT_seed))
    vs = float(np.asarray(inputs["vss_skip"]).ravel()[0])
    cvm = float(np.asarray(inputs["cvm_skip"]).ravel()[0])
    return in_maps, vs, cvm


def kernel(**inputs) -> np.ndarray:
    from concourse.bass_utils import run_bass_kernel_spmd

    in_maps, vs, cvm = _host_prep(inputs)
    key = (vs, cvm)
    if key not in _cache:
        _cache[key] = _build(vs, cvm)
    nc = _cache[key]
    res = run_bass_kernel_spmd(nc, in_maps, core_ids=list(range(NCORES)))
    out = np.zeros((B, H, W, C_IN), np.float32)
    for b in range(B):
        full = np.zeros((C_IN, L), np.float32)
        for r in range(4):
            full[:, r * SHARD:(r + 1) * SHARD] = \
                res.results[4 * b + r]["out_cf"]
        out[b] = full.T.reshape(H, W, C_IN)
    return out


# revision 22
# speedup vs baseline: 1.4380x; 1.1143x over previous
"""Trainium2 Bass kernel for nn_CascadedVMambaBlock (v2).

Sharding: 8 cores; core c = (b, nh) with b = c//4, nh = c%4.
Each core processes sample b with state-dim slice n in [4nh, 4nh+4)
for ALL 4 scan directions k. Per-head combine is a ReduceScatter over
the 4-core b-group (each rank keeps an L-shard of 576 pixels), the
out_norm/out_proj tail runs sharded, and an AllGather rebuilds the
full next-stage input.

Layouts: channels-first (channels on partitions, L = 2304 free).
Scan lanes are INTERLEAVED: partition p = 2*d + half covers channel
d = p//2, state n = 4*nh + 2*gl + half.

v2 structure (vs v1):
- B/C row-expansion: the x_proj matmul emits a [128,512] seed whose
  output rows repeat the 8-row (B0h0,B0h1,B1h0,B1h1,C0..C1..) pattern
  16x (PE cost is free-dim only), one scalar copy -> fp16 seed tile,
  then per (gl,tensor) just 3 chained SBUF DMAs (32 rows gathered,
  then 2 log-doublings) instead of a 7-deep chain.
- Scan-prep (dt matmul, softplus, u2, dA, B/C) is computed full-L per
  direction k, 1-2 directions ahead, so the DVE runs the scan phase
  as one op per (k,gl) with no intra-k chaining.
- xs_cm is written directly from conv PSUM by a second strided silu;
  cm ys chunks write the h-major transpose (y_cmg) directly.
- KORD = (0,2,1,3): the rm ReduceScatter is issued mid-head and mostly
  hidden; only the cm ReduceScatter (y + sz piggyback) is exposed.
"""
import numpy as np

HEAD, C_IN, C_H = 4, 128, 32
D, N, K, DT_RANK = 64, 16, 4, 2
B, H, W = 2, 48, 48
L = H * W            # 2304
CS = 480             # LN1 chunk (10 image rows -> canvas-aligned)
LNCH = [(i * CS, min(CS, L - i * CS)) for i in range((L + CS - 1) // CS)]
SHARD = L // 4       # 576 pixels per rank
HCH = SHARD // 2     # 288-wide half-shard chunks
PADW = 50 * 51       # padded conv canvas (extra tail row for AP slack)
EPS = 1e-5
NCORES = 8
KORD = (0, 2, 1, 3)

_cache = {}


DEBUG = False


def _build(vs, cvm):
    import concourse.bass as bass
    import concourse.bacc as bacc
    import concourse.tile as tile
    import concourse.mybir as mybir
    from contextlib import ExitStack

    f32 = mybir.dt.float32
    fp16 = mybir.dt.float16
    AF = mybir.ActivationFunctionType
    OP = mybir.AluOpType

    import concourse.hw_specs as hw_specs
    _orig_gat = hw_specs.get_activation_tables
    _KEEP = {"natural_log_exp_and_others", "silu_and_others"}

    def _patched_gat(arch):
        t = _orig_gat(arch)
        return {k: (v if k in _KEEP else set()) for k, v in t.items()}

    bacc.get_activation_tables = _patched_gat

    nc = bacc.Bacc("TRN2", target_bir_lowering=False, debug=False,
                   enable_asserts=True, num_devices=NCORES)

    def din(name, shape, dtype=f32):
        return nc.dram_tensor(name, shape, dtype, kind="ExternalInput").ap()

    x_shuf_d = din("x_shuf", (C_IN, L), fp16)
    x_my_d = din("x_my", (C_IN, SHARD))
    lhsT_cv_d = din("lhsT_cv", (99, 3, C_IN), fp16)   # conv, il-duplicated out
    lhsT_z_d = din("lhsT_z", (99, D), fp16)           # z half (dy=1 only)
    bias_cv_d = din("bias_cv", (C_IN, 1))             # silu bias, il-duplicated
    bias_z_d = din("bias_z", (D, 1))
    x_myr_d = din("x_my_r", (C_H, 4 * SHARD), fp16)
    lhsT_sq16_d = din("lhsT_sq16", (C_H, 1), fp16)
    lhsT_y1_d = din("lhsT_y1", (D, 1), fp16)
    lhsT_bc_d = din("lhsT_bc", (1, C_IN))
    lhsT_M2_d = din("lhsT_M2", (C_IN, K, C_IN), fp16)
    dtb2_d = din("dtb2", (C_IN, K))
    A2_d = din("A2", (C_IN, K, 2))
    lhsT_seed_d = din("lhsT_seed", (C_IN, K, C_IN), fp16)
    lhsT_ys_d = din("lhsT_ys", (C_IN, D), fp16)
    lhsT_ds_d = din("lhsT_ds", (C_IN, D), fp16)       # Ds init (k=0 psum)
    lhsT_op_d = din("lhsT_op", (D, C_H), fp16)
    lhsT_stf_d = din("lhsT_stf", (C_IN, 2), fp16)
    gamma_d = din("gamma", (C_IN, 1))
    beta_d = din("beta", (C_IN, 1))

    out_d = nc.dram_tensor("out_cf", (C_IN, SHARD), f32,
                           kind="ExternalOutput").ap()
    dbg_d = {}
    if DEBUG:
        for nm, shp in (("xs_il", (C_IN, L)), ("xs_cm", (C_IN, L)),
                        ("sz", (D, L)), ("seed0", (C_IN, L)),
                        ("B0k0", (C_IN, L)), ("C1k0", (C_IN, L)),
                        ("dt2k0", (C_IN, L)), ("u2k0", (C_IN, L)),
                        ("dA0k0", (C_IN, L)), ("h0k0", (C_IN, L)),
                        ("y_rm", (D, L)), ("y_cmg", (D, L)),
                        ("srctr", (C_H, PADW)), ("mrctr", (1, PADW))):
            dbg_d[nm] = nc.dram_tensor("dbg_" + nm, shp, fp16,
                                       kind="ExternalOutput").ap()

    RG = [[0, 1, 2, 3], [4, 5, 6, 7]]

    with tile.TileContext(nc) as tc, ExitStack() as ctx:
        w_pool = ctx.enter_context(tc.tile_pool(name="weights", bufs=1))
        big = ctx.enter_context(tc.tile_pool(name="big", bufs=1))
        stg = ctx.enter_context(tc.tile_pool(name="stg", bufs=1))
        sml = ctx.enter_context(tc.tile_pool(name="sml", bufs=2))
        scn = ctx.enter_context(tc.tile_pool(name="scn", bufs=2))
        ps = ctx.enter_context(tc.tile_pool(name="ps", bufs=1, space="PSUM"))
        dram = ctx.enter_context(tc.tile_pool(name="dram", bufs=2, space="DRAM"))

        def wload(ap_d, shape, dtype=f32):
            t = w_pool.tile(list(shape), dtype, name=ap_d.tensor.name + "_sb")
            src = ap_d if ap_d.dtype == dtype else ap_d.bitcast(dtype)
            nc.sync.dma_start(t[:], src)
            return t

        x_my = wload(x_my_d, (C_IN, SHARD))
        lhsT_cv = wload(lhsT_cv_d, (99, 3, C_IN), fp16)
        lhsT_z = wload(lhsT_z_d, (99, D), fp16)
        bias_cv = wload(bias_cv_d, (C_IN, 1))
        bias_z = wload(bias_z_d, (D, 1))
        x_my_r = wload(x_myr_d, (C_H, 4 * SHARD), fp16)
        lhsT_sq16 = wload(lhsT_sq16_d, (C_H, 1), fp16)
        lhsT_y1 = wload(lhsT_y1_d, (D, 1), fp16)
        lhsT_bc = wload(lhsT_bc_d, (1, C_IN))
        lhsT_M2 = wload(lhsT_M2_d, (C_IN, K, C_IN), fp16)
        dtb2 = wload(dtb2_d, (C_IN, K))
        A2 = wload(A2_d, (C_IN, K, 2))
        lhsT_seed = wload(lhsT_seed_d, (C_IN, K, C_IN), fp16)
        lhsT_ys = wload(lhsT_ys_d, (C_IN, D), fp16)
        lhsT_ds = wload(lhsT_ds_d, (C_IN, D), fp16)
        lhsT_op = wload(lhsT_op_d, (D, C_H), fp16)
        lhsT_stf = wload(lhsT_stf_d, (C_IN, 2), fp16)
        gamma = wload(gamma_d, (C_IN, 1))
        beta = wload(beta_d, (C_IN, 1))

        # persistent tiles
        # canvas rows: 0:32 sr(dx0), 32:64 sr(center/dx1), 64:96 sr(dx2),
        #              96 mr(center/dx1), 97 mr(dx0), 98 mr(dx2)
        # (mr center must sit on a quarter-aligned partition for DVE writes)
        sr3 = big.tile([99, PADW], fp16)
        nc.vector.memset(sr3[:], 0.0)
        xs_il = big.tile([C_IN, L], fp16)    # duplicated-interleaved xs (rm)
        xs_cm = big.tile([C_IN, L], fp16)    # col-major layout (from conv)
        outs_sh = big.tile([C_IN, SHARD], f32)
        prev_full = big.tile([C_H, L], fp16)
        y_rm = big.tile([D, L], fp16)
        y_cmg = big.tile([D, L], fp16)       # cm result, already h-major
        sz = big.tile([D, L], fp16)          # silu(z), full L

        # canvas views
        CTR = sr3[32:64, :]

        def cview(rows, j, nr):
            # canvas view writing sr[h, w] at position 51 + 50h + w: the
            # CENTER region is left-shifted by one col (R_1[h, b] = sr[h, b+1])
            base = 51 + 500 * j
            return rows[:, base:base + 50 * nr].rearrange(
                "c (h w) -> c h w", h=nr, w=50)[:, :, 0:48]

        def ln_smalls(ps_me, w, tagp):
            # ps_me psum: row 0 = mean, row 32 = E[x^2]
            m2_c = sml.tile([1, CS], f32, tag=tagp + "m2", name="m2_c")
            nc.scalar.square(m2_c[:, :w], ps_me[0:1, :w])
            var_c = sml.tile([1, CS], f32, tag=tagp + "v", name="var_c")
            nc.vector.scalar_tensor_tensor(var_c[:, :w], ps_me[32:33, :w],
                                           EPS, m2_c[:, :w], OP.add,
                                           OP.subtract)
            lnv_c = sml.tile([1, CS], f32, tag=tagp + "m2", name="lnv_c")
            nc.scalar.activation(lnv_c[:, :w], var_c[:, :w], AF.Ln)
            r_c = sml.tile([1, CS], f32, tag=tagp + "r", name="r_c")
            nc.scalar.activation(r_c[:, :w], lnv_c[:, :w], AF.Exp, scale=-0.5)
            return r_c

        s_t = None
        prev_sh_ap = None
        for i in range(HEAD):
            # ---- stage input s (32, L) ----
            chunk_sb = sml.tile([C_H, L], fp16, tag="s_cs", name="chunk_sb",
                                bufs=1)
            nc.sync.dma_start(chunk_sb[:], x_shuf_d[32 * i:32 * (i + 1), :])
            if i == 0:
                s_t = chunk_sb[:]
            else:
                nc.vector.tensor_add(prev_full[:], prev_full[:], chunk_sb[:])
                s_t = prev_full[:]

            # ---- LN1: stats + sr/mr written into canvas center ----
            for ci, (o, w) in enumerate(LNCH):
                nr = w // 48
                sq_c = sml.tile([C_H, CS], fp16, tag="sqc", name="sq_c")
                nc.scalar.square(sq_c[:, :w], s_t[:, o:o + w])
                ps_me = ps.tile([33, 512], f32, tag="st", name="ps_me",
                                bufs=1)
                nc.tensor.matmul(ps_me[0:1, :w], lhsT_sq16[:],
                                 s_t[:, o:o + w], start=True, stop=True)
                nc.tensor.matmul(ps_me[32:33, :w], lhsT_sq16[:],
                                 sq_c[:, :w], start=True, stop=True)
                r_c = ln_smalls(ps_me, w, "sm")
                # mr -> canvas row 96 (center), strided over rows
                nc.vector.tensor_mul(
                    cview(sr3[96:97, :], ci, nr), ps_me[0:1, :w].rearrange(
                        "c (h w) -> c h w", h=nr, w=48),
                    r_c[:, :w].rearrange("c (h w) -> c h w", h=nr, w=48))
                ps_rr = ps.tile([C_IN, 512], f32, tag="sp", name="ps_rr",
                                bufs=2)
                nc.tensor.matmul(ps_rr[0:C_H, :w], lhsT_bc[0:1, 0:C_H],
                                 r_c[:, :w], start=True, stop=True)
                nc.vector.tensor_mul(
                    cview(CTR, ci, nr),
                    s_t[:, o:o + w].rearrange("c (h w) -> c h w", h=nr, w=48),
                    ps_rr[0:C_H, :w].rearrange("c (h w) -> c h w", h=nr, w=48))

            # ---- dx-shifted copies (center -> dx0/dx2 regions) ----
            ctr_flat = sr3[32:64, 51:51 + 2400].rearrange(
                "c (h w) -> c h w", h=48, w=50)
            mr_flat = sr3[96:97, 51:51 + 2400].rearrange(
                "c (h w) -> c h w", h=48, w=50)
            for dx in (0, 2):
                b0 = max(0, dx - 1)
                b1 = min(47, 46 + dx)          # inclusive src col range
                wdt = b1 - b0 + 1
                base = 52 + b0 - dx
                mrrow = 97 if dx == 0 else 98
                dv = sr3[32 * dx:32 * dx + 32, base:base + 2400].rearrange(
                    "c (h w) -> c h w", h=48, w=50)[:, :, 0:wdt]
                nc.sync.dma_start(dv, ctr_flat[:, :, b0:b1 + 1])
                mv = sr3[mrrow:mrrow + 1, base:base + 2400].rearrange(
                    "c (h w) -> c h w", h=48, w=50)[:, :, 0:wdt]
                nc.sync.dma_start(mv, mr_flat[:, :, b0:b1 + 1])

            # ---- conv/in_proj: accumulated matmuls; dual silu out ----
            h0 = 0
            while h0 < 48:
                nr = min(10, 48 - h0)
                wch = nr * 48
                ps_cv = ps.tile([C_IN, 512], f32, tag="cv", name="ps_cv",
                                bufs=1)
                pv = ps_cv[:, :wch].rearrange("c (h w) -> c h w", h=nr, w=48)
                ps_z = ps.tile([D, 512], f32, tag="ys", name="ps_z", bufs=2)
                zv = ps_z[:, :wch].rearrange("c (h w) -> c h w", h=nr, w=48)
                for dy in range(3):
                    base = 1 + 50 * dy + 50 * h0
                    rv = sr3[:, base:base + 50 * nr].rearrange(
                        "c (h w) -> c h w", h=nr, w=50)[:, :, 0:48]
                    nc.tensor.matmul(pv, lhsT_cv[:, dy, :], rv,
                                     start=(dy == 0), stop=(dy == 2))
                    if dy == 1:
                        nc.tensor.matmul(zv, lhsT_z[:], rv,
                                         start=True, stop=True)
                nc.scalar.activation(sz[:, 48 * h0:48 * h0 + wch],
                                     ps_z[:, :wch], AF.Silu, bias=bias_z[:])
                nc.scalar.activation(xs_il[:, 48 * h0:48 * h0 + wch],
                                     ps_cv[:, :wch], AF.Silu, bias=bias_cv[:])
                # second silu: write col-major layout directly
                cm_dst = xs_cm[:].rearrange(
                    "c (w h) -> c w h", w=48, h=48)[:, :, h0:h0 + nr]
                nc.scalar.activation(
                    cm_dst, ps_cv[:, :wch].rearrange(
                        "c (h w) -> c w h", h=nr, w=48),
                    AF.Silu, bias=bias_cv[:])
                h0 += nr

            # ---- scan preps + scans ----
            def prep_bc(k):
                # seed matmul + copies, then depth-2 expansion DMA chains;
                # independent of the dt/dA chain, so emitted early
                xs2 = xs_il if k in (0, 2) else xs_cm
                seed = scn.tile([C_IN, L], fp16, tag="seed", name="seed",
                                bufs=2)
                for o in range(0, L, 512):
                    sw = min(512, L - o)
                    ps_sd = ps.tile([C_IN, 512], f32, tag="sp", name="ps_sd",
                                    bufs=2)
                    nc.tensor.matmul(ps_sd[:, :sw], lhsT_seed[:, k, :],
                                     xs2[:, o:o + sw], start=True, stop=True)
                    nc.scalar.copy(seed[:, o:o + sw], ps_sd[:, :sw])
                # SBUF APs may only stride partitions in dim 0, so gather
                # each parity separately; 4 parallel gathers + 1 doubling
                BC = {}
                for ti, tag in enumerate(("B0", "B1", "C0", "C1")):
                    t0 = (ti % 2) * 2 + (ti // 2) * 4
                    dstt = scn.tile([C_IN, L], fp16, tag=tag, name=tag,
                                    bufs=2)
                    for half in range(2):
                        for t in range(2):
                            nc.sync.dma_start(
                                dstt[32 * half + t:32 * half + 32:2, :],
                                seed[t0 + t:C_IN:8, :][0:16, :])
                    nc.sync.dma_start(dstt[64:128, :], dstt[0:64, :])
                    BC[tag] = dstt
                return dict(xs2=xs2, BC=BC, seed_t=seed)

            def prep_dt(k, P):
                xs2 = P["xs2"]
                ech = scn.tile([C_IN, L], fp16, tag="ech", name="ech", bufs=1)
                for o in range(0, L, 1024):
                    cw = min(1024, L - o)
                    ps_dt = ps.tile([C_IN, 1024], f32, tag="pdt",
                                    name="ps_dt", bufs=1)
                    for so in range(0, cw, 512):
                        sw = min(512, cw - so)
                        nc.tensor.matmul(ps_dt[:, so:so + sw],
                                         lhsT_M2[:, k, :],
                                         xs2[:, o + so:o + so + sw],
                                         start=True, stop=True)
                    nc.scalar.activation(ech[:, o:o + cw], ps_dt[:, :cw],
                                         AF.Exp, bias=dtb2[:, k:k + 1])
                dt2 = scn.tile([C_IN, L], fp16, tag="dt2", name="dt2", bufs=1)
                nc.scalar.activation(dt2[:], ech[:], AF.Ln, bias=1.0)
                P["dt2"] = dt2
                dAs = []
                for gl in range(2):
                    dA = scn.tile([C_IN, L], fp16, tag=f"dA{gl}", name="dA",
                                  bufs=2)
                    nc.scalar.activation(dA[:], dt2[:], AF.Exp,
                                         scale=A2[:, k, gl:gl + 1])
                    dAs.append(dA)
                P["dAs"] = dAs
                return P

            def scan(k, P):
                rev = k >= 2
                u2 = scn.tile([C_IN, L], fp16, tag="u2", name="u2", bufs=1)
                nc.vector.tensor_mul(u2[:], P["dt2"][:], P["xs2"][:])
                hCs = []
                for gl in range(2):
                    Bt = P["BC"]["B0" if gl == 0 else "B1"]
                    Ct = P["BC"]["C0" if gl == 0 else "C1"]
                    bB = scn.tile([C_IN, L], fp16, tag="bB", name="bB",
                                  bufs=2)
                    nc.vector.tensor_mul(bB[:], u2[:], Bt[:])
                    h_c = scn.tile([C_IN, L], fp16, tag="h", name="h_c",
                                   bufs=2)
                    if not rev:
                        nc.vector.tensor_tensor_scan(
                            h_c[:], P["dAs"][gl][:], bB[:], 0.0,
                            OP.mult, OP.add)
                    else:
                        nc.vector.tensor_tensor_scan(
                            h_c[:][:, ::-1], P["dAs"][gl][:][:, ::-1],
                            bB[:][:, ::-1], 0.0, OP.mult, OP.add)
                    hC = scn.tile([C_IN, L], fp16, tag="hC", name="hC",
                                  bufs=2)
                    nc.vector.tensor_mul(hC[:], h_c[:], Ct[:])
                    hCs.append(hC)
                # ys reduction
                if k in (0, 2):        # rm pair -> y_rm (row-major)
                    for o in range(0, L, 512):
                        sw = min(512, L - o)
                        ps_ys = ps.tile([D, 512], f32, tag="ys", name="ps_ys",
                                        bufs=2)
                        if k == 0:
                            nc.tensor.matmul(ps_ys[:, :sw], lhsT_ds[:],
                                             xs_il[:, o:o + sw],
                                             start=True, stop=False,
                                             skip_group_check=True)
                        for gl in range(2):
                            nc.tensor.matmul(ps_ys[:, :sw], lhsT_ys[:],
                                             hCs[gl][:, o:o + sw],
                                             start=(gl == 0 and k != 0),
                                             stop=(gl == 1),
                                             skip_group_check=True)
                        if k == 0:
                            nc.scalar.copy(y_rm[:, o:o + sw], ps_ys[:, :sw])
                        else:
                            nc.vector.tensor_add(y_rm[:, o:o + sw],
                                                 y_rm[:, o:o + sw],
                                                 ps_ys[:, :sw])
                else:                  # cm pair -> y_cmg (h-major direct)
                    for j in range(5):
                        o = 480 * j
                        sw = min(480, L - o)
                        nw = sw // 48
                        ps_ys = ps.tile([D, 512], f32, tag="ys", name="ps_ys",
                                        bufs=2)
                        for gl in range(2):
                            nc.tensor.matmul(ps_ys[:, :sw], lhsT_ys[:],
                                             hCs[gl][:, o:o + sw],
                                             start=(gl == 0), stop=(gl == 1),
                                             skip_group_check=True)
                        # y_cm chunk o covers w-cols [10j, 10j+nw): write
                        # transposed into y_cmg (h-major)
                        dstv = y_cmg[:].rearrange(
                            "c (h w) -> c h w", h=48, w=48)[:, :, 10 * j:
                                                            10 * j + nw]
                        srcv = ps_ys[:, :sw].rearrange(
                            "c (w h) -> c h w", w=nw, h=48)
                        if k == 1:
                            nc.scalar.copy(dstv, srcv)
                        else:
                            nc.vector.tensor_add(dstv, dstv, srcv)

            P1 = prep_bc(1)
            P3 = prep_bc(3)
            prep_dt(1, P1)
            prep_dt(3, P3)
            scan(1, P1)
            P0 = prep_bc(0)
            prep_dt(0, P0)
            scan(3, P3)

            # ---- cm ReduceScatter (y_cmg + sz piggyback), issued mid-head
            ci_cm = dram.tile([4 * C_IN, SHARD], fp16, tag="rs_cm_in",
                              name="ci_cm", bufs=2)
            co_cm = dram.tile([C_IN, SHARD], fp16, tag="rs_cm_out",
                              name="co_cm", bufs=2)
            civ = ci_cm[:].rearrange("(r c) s -> c r s", r=4, c=C_IN)
            nc.sync.dma_start(
                civ[0:D], y_cmg[:].rearrange("c (r s) -> c r s", r=4,
                                             s=SHARD))
            nc.sync.dma_start(
                civ[D:C_IN], sz[:].rearrange("c (r s) -> c r s", r=4,
                                             s=SHARD))
            nc.gpsimd.collective_compute(
                "ReduceScatter", OP.add, replica_groups=RG,
                ins=[ci_cm[:].opt()], outs=[co_cm[:].opt()])

            P2 = prep_bc(2)
            prep_dt(2, P2)
            scan(0, P0)
            scan(2, P2)

            # ---- rm ReduceScatter (smaller payload; exposed at tail) ----
            ri = dram.tile([4 * D, SHARD], fp16, tag="rs_rm_in", name="ri",
                           bufs=2)
            ro = dram.tile([D, SHARD], fp16, tag="rs_rm_out", name="ro",
                           bufs=2)
            nc.sync.dma_start(
                ri[:].rearrange("(r c) s -> c r s", r=4, c=D),
                y_rm[:].rearrange("c (r s) -> c r s", r=4, s=SHARD))
            nc.gpsimd.collective_compute(
                "ReduceScatter", OP.add, replica_groups=RG,
                ins=[ri[:].opt()], outs=[ro[:].opt()])

            # ---- sharded tail ----
            cm_sh = sml.tile([D, SHARD], fp16, tag="cm_sh", name="cm_sh")
            nc.sync.dma_start(cm_sh[:], co_cm[0:D, :])
            sz_sh = sml.tile([D, SHARD], fp16, tag="sz_sh", name="sz_sh")
            nc.sync.dma_start(sz_sh[:], co_cm[D:C_IN, :])
            rm_sh = sml.tile([D, SHARD], fp16, tag="rm_sh", name="rm_sh")
            nc.sync.dma_start(rm_sh[:], ro[:])
            if i == 0:
                s_sh = x_my_r[:, 0:SHARD]
            else:
                s_sh_t = sml.tile([C_H, SHARD], f32, tag="s_sh", name="s_sh")
                nc.vector.tensor_add(s_sh_t[:], prev_sh_ap,
                                     x_my_r[:, i * SHARD:(i + 1) * SHARD])
                s_sh = s_sh_t[:]
            y_sh = sml.tile([D, SHARD], fp16, tag="y_sh", name="y_sh")
            nc.vector.tensor_add(y_sh[:], rm_sh[:], cm_sh[:])
            ysq_t = sml.tile([D, SHARD], fp16, tag="ysqt", name="ysq_t")
            nc.scalar.square(ysq_t[:], y_sh[:])
            prev_sh = sml.tile([C_H, SHARD], f32, tag="prevsh",
                               name="prev_sh")
            for hh in range(2):
                o = hh * HCH
                ps_me2 = ps.tile([33, 512], f32, tag="st", name="ps_me2",
                                 bufs=1)
                nc.tensor.matmul(ps_me2[0:1, :HCH], lhsT_y1[:],
                                 y_sh[:, o:o + HCH], start=True, stop=True)
                nc.tensor.matmul(ps_me2[32:33, :HCH], lhsT_y1[:],
                                 ysq_t[:, o:o + HCH], start=True, stop=True)
                r_c = ln_smalls(ps_me2, HCH, "sm")
                m_c = sml.tile([1, HCH], f32, tag="mct", name="m_c")
                nc.scalar.copy(m_c[:], ps_me2[0:1, :HCH])
                ps_mb = ps.tile([D, 512], f32, tag="ys", name="ps_mb",
                                bufs=2)
                nc.tensor.matmul(ps_mb[:, :HCH], lhsT_bc[0:1, 0:D],
                                 m_c[:], start=True, stop=True)
                ps_rb = ps.tile([C_IN, 512], f32, tag="sp", name="ps_rb",
                                bufs=2)
                nc.tensor.matmul(ps_rb[0:C_H, :HCH], lhsT_bc[0:1, 0:C_H],
                                 r_c[:, :HCH], start=True, stop=True)
                ym = sml.tile([D, HCH], f32, tag="ym", name="ym")
                nc.vector.tensor_sub(ym[:], y_sh[:, o:o + HCH],
                                     ps_mb[:, :HCH])
                ysz = sml.tile([D, HCH], fp16, tag="ysz", name="ysz")
                nc.vector.tensor_mul(ysz[:], ym[:], sz_sh[:, o:o + HCH])
                ps_op = ps.tile([C_IN, 512], f32, tag="sp", name="ps_op",
                                bufs=2)
                nc.tensor.matmul(ps_op[0:C_H, :HCH], lhsT_op[:], ysz[:],
                                 start=True, stop=True)
                op_sb = sml.tile([C_H, HCH], f32, tag="op_sb", name="op_sb")
                nc.scalar.copy(op_sb[:], ps_op[0:C_H, :HCH])
                t_c = sml.tile([C_H, HCH], f32, tag="t_c", name="t_c")
                nc.vector.tensor_mul(t_c[:], op_sb[:], ps_rb[0:C_H, :HCH])
                nc.vector.scalar_tensor_tensor(
                    prev_sh[:, o:o + HCH],
                    s_sh[:, o:o + HCH], 1.0 + vs, t_c[:], OP.mult, OP.add)
            nc.sync.dma_start(outs_sh[32 * i:32 * (i + 1), :], prev_sh[:])
            prev_sh_ap = prev_sh[:]

            if i < HEAD - 1:
                prev16 = sml.tile([C_H, SHARD], fp16, tag="prev16",
                                  name="prev16")
                nc.scalar.copy(prev16[:], prev_sh[:])
                agi = dram.tile([C_H, SHARD], fp16, tag="ag_in", name="agi",
                                bufs=2)
                ago = dram.tile([4 * C_H, SHARD], fp16, tag="ag_out",
                                name="ago", bufs=2)
                nc.sync.dma_start(agi[:], prev16[:])
                nc.gpsimd.collective_compute(
                    "AllGather", OP.bypass, replica_groups=RG,
                    ins=[agi[:].opt()], outs=[ago[:].opt()])
                nc.sync.dma_start(
                    prev_full[:].rearrange("c (r s) -> c r s", r=4, s=SHARD),
                    ago[:].rearrange("(r c) s -> c r s", r=4, c=C_H))

        # ---- final: x_res = cvm*x_my + outs_sh (shard); LN over 128 ch ----
        xres = stg.tile([C_IN, SHARD], f32, tag="xres", name="xres")
        nc.vector.scalar_tensor_tensor(xres[:], x_my[:], cvm,
                                       outs_sh[:], OP.mult, OP.add)
        x16 = stg.tile([C_IN, SHARD], fp16, tag="x16", name="x16")
        nc.scalar.copy(x16[:], xres[:])
        xsq = stg.tile([C_IN, SHARD], fp16, tag="xsq", name="xsq")
        nc.scalar.square(xsq[:], xres[:])
        out_sb = stg.tile([C_IN, SHARD], f32, tag="out_sb", name="out_sb")
        for hh in range(2):
            o = hh * HCH
            ps_me3 = ps.tile([33, 512], f32, tag="st", name="ps_me3",
                             bufs=1)
            nc.tensor.matmul(ps_me3[0:1, :HCH], lhsT_stf[:, 0:1],
                             x16[:, o:o + HCH], start=True, stop=True)
            nc.tensor.matmul(ps_me3[32:33, :HCH], lhsT_stf[:, 1:2],
                             xsq[:, o:o + HCH], start=True, stop=True)
            r_c = ln_smalls(ps_me3, HCH, "sm")
            mr_c = sml.tile([1, HCH], f32, tag="smv", name="mr_c3")
            nc.vector.tensor_mul(mr_c[:], ps_me3[0:1, :HCH], r_c[:, :HCH])
            ps_ra = ps.tile([C_IN, 512], f32, tag="sp", name="ps_ra3",
                            bufs=2)
            nc.tensor.matmul(ps_ra[:, :HCH], lhsT_bc[:], r_c[:, :HCH],
                             start=True, stop=True)
            ps_rb = ps.tile([C_IN, 512], f32, tag="sp", name="ps_rb3",
                            bufs=2)
            nc.tensor.matmul(ps_rb[:, :HCH], lhsT_bc[:], mr_c[:],
                             start=True, stop=True)
            nc.vector.tensor_mul(out_sb[:, o:o + HCH], xres[:, o:o + HCH],
                                 ps_ra[:, :HCH])
            nc.vector.tensor_sub(out_sb[:, o:o + HCH], out_sb[:, o:o + HCH],
                                 ps_rb[:, :HCH])
            nc.vector.tensor_scalar(out_sb[:, o:o + HCH],
                                    out_sb[:, o:o + HCH],
                                    gamma[:], beta[:], OP.mult, OP.add)
        nc.sync.dma_start(out_d[:], out_sb[:])

    nc.compile()
    return nc


def _host_prep(inputs):
    """Build per-core input maps from full inputs."""
    fp16 = np.float16
    x = np.asarray(inputs["x"], np.float32)
    ln1_w = np.asarray(inputs["ln1_w"], np.float32)
    ln1_b = np.asarray(inputs["ln1_b"], np.float32)
    in_proj_w = np.asarray(inputs["in_proj_w"], np.float32)
    conv_w = np.asarray(inputs["conv_w"], np.float32)
    conv_b = np.asarray(inputs["conv_b"], np.float32)
    x_proj_w = np.asarray(inputs["x_proj_w"], np.float32)
    dt_proj_w = np.asarray(inputs["dt_proj_w"], np.float32)
    dt_proj_b = np.asarray(inputs["dt_proj_b"], np.float32)
    A_logs = np.asarray(inputs["A_logs"], np.float32)
    Ds = np.asarray(inputs["Ds"], np.float32)
    out_norm_w = np.asarray(inputs["out_norm_w"], np.float32)
    out_norm_b = np.asarray(inputs["out_norm_b"], np.float32)
    out_proj_w = np.asarray(inputs["out_proj_w"], np.float32)
    final_ln_w = np.asarray(inputs["final_ln_w"], np.float32)
    final_ln_b = np.asarray(inputs["final_ln_b"], np.float32)
    assert not np.any(out_norm_b), "out_norm_b must be zero (folded)"

    Wxx = (in_proj_w[0:D] * ln1_w[None, :]).astype(np.float32)    # (64, 32)
    Wz = (in_proj_w[D:C_IN] * ln1_w[None, :]).astype(np.float32)  # (64, 32)
    bias_ip = (in_proj_w @ ln1_b).astype(np.float32)              # (128,)
    w9 = np.ascontiguousarray(
        conv_w[:, :, 0, :].transpose(2, 0, 1).reshape(D, 9))      # (64, 9)
    colsum_xx = Wxx.sum(1)
    colsum_z = Wz.sum(1)

    # conv lhsT with duplicated-interleaved output: out col p = 2d+half
    lhsT_cv = np.zeros((99, 3, C_IN), fp16)
    for dy in range(3):
        for dx in range(3):
            tap = 3 * dy + dx
            blk = (w9[:, tap][None, :] * Wxx.T)          # (32c, 64d)
            cor = (-w9[:, tap] * colsum_xx)              # (64d,)
            mrrow = 96 if dx == 1 else (97 if dx == 0 else 98)
            for half in range(2):
                lhsT_cv[32 * dx:32 * dx + 32, dy, half::2] = blk.astype(fp16)
                lhsT_cv[mrrow, dy, half::2] = cor.astype(fp16)
    lhsT_z = np.zeros((99, D), fp16)
    lhsT_z[32:64, :] = Wz.T.astype(fp16)
    lhsT_z[96, :] = (-colsum_z).astype(fp16)

    bias_cv = np.zeros((C_IN, 1), np.float32)
    bcv = conv_b + w9.sum(1) * bias_ip[0:D]
    bias_cv[0::2, 0] = bcv
    bias_cv[1::2, 0] = bcv
    bias_z = bias_ip[D:C_IN].reshape(D, 1).astype(np.float32)

    lhsT_sq16 = np.full((C_H, 1), 1.0 / C_H, fp16)
    lhsT_y1 = np.full((D, 1), 1.0 / D, fp16)
    lhsT_bc = np.ones((1, C_IN), np.float32)

    # dt projection: contraction over il rows (2c+half -> /2), out il cols
    M = np.einsum("kdr,krc->kdc", dt_proj_w, x_proj_w[:, :DT_RANK, :])
    lhsT_M2 = np.zeros((C_IN, K, C_IN), fp16)
    dtb2 = np.zeros((C_IN, K), np.float32)
    for k in range(K):
        mh = (M[k].T / 2.0).astype(fp16)
        for half_in in range(2):
            for half_out in range(2):
                lhsT_M2[half_in::2, k, half_out::2] = mh
        dtb2[0::2, k] = dt_proj_b[k, :]
        dtb2[1::2, k] = dt_proj_b[k, :]

    A = -np.exp(A_logs)                                  # (K, 64, 16)
    Ds_q = (Ds.sum(0) / 4.0).astype(np.float32)          # (64,)
    # out_proj lhsT; includes out_norm gamma fold and the /4 for the sz RS
    W_op = (out_proj_w * out_norm_w[None, :]) / 4.0
    lhsT_op = np.ascontiguousarray(W_op.T).astype(fp16)  # (64, 32)

    lhsT_ys = np.zeros((C_IN, D), fp16)
    lhsT_ds = np.zeros((C_IN, D), fp16)
    for d in range(D):
        lhsT_ys[2 * d, d] = 1.0
        lhsT_ys[2 * d + 1, d] = 1.0
        lhsT_ds[2 * d, d] = Ds_q[d] / 2.0
        lhsT_ds[2 * d + 1, d] = Ds_q[d] / 2.0
    lhsT_stf = np.zeros((C_IN, 2), fp16)
    lhsT_stf[:, 0] = 1.0 / C_IN
    lhsT_stf[:, 1] = 1.0 / C_IN

    common = {
        "lhsT_cv": lhsT_cv, "lhsT_z": lhsT_z,
        "bias_cv": bias_cv, "bias_z": bias_z,
        "lhsT_sq16": lhsT_sq16,
        "lhsT_y1": lhsT_y1, "lhsT_bc": lhsT_bc,
        "lhsT_M2": lhsT_M2, "dtb2": dtb2,
        "lhsT_ys": lhsT_ys, "lhsT_ds": lhsT_ds,
        "lhsT_op": lhsT_op,
        "lhsT_stf": lhsT_stf,
        "gamma": final_ln_w.reshape(C_IN, 1),
        "beta": final_ln_b.reshape(C_IN, 1),
    }

    g = HEAD
    cg = C_IN // HEAD
    per_b = []
    per_b32 = []
    for b in range(B):
        xs = x[b].reshape(H, W, g, cg).transpose(0, 1, 3, 2).reshape(L, C_IN)
        cf = np.ascontiguousarray(xs.T)
        per_b32.append(cf)
        per_b.append(cf.astype(fp16))  # (128, L)

    in_maps = []
    for c in range(NCORES):
        b, nh = c // 4, c % 4
        A2 = np.zeros((C_IN, K, 2), np.float32)
        lhsT_seed = np.zeros((C_IN, K, C_IN), fp16)
        for k in range(K):
            for gl in range(2):
                for half in range(2):
                    n = 4 * nh + 2 * gl + half
                    for d in range(D):
                        A2[2 * d + half, k, gl] = A[k, d, n]
            # seed pattern: out col q = 8*j + t; t: 0,1=B gl0 h0,h1;
            # 2,3=B gl1; 4,5=C gl0; 6,7=C gl1
            for t in range(8):
                is_c = t // 4
                gl = (t % 4) // 2
                half = t % 2
                n = 4 * nh + 2 * gl + half
                row = x_proj_w[k, DT_RANK + 16 * is_c + n, :] / 2.0
                for half_in in range(2):
                    lhsT_seed[half_in::2, k, t::8] = \
                        row.astype(fp16)[:, None]
        x_shuf_cf = per_b[b]
        x_my = np.ascontiguousarray(
            per_b32[b][:, nh * SHARD:(nh + 1) * SHARD])
        x_my_r = np.zeros((C_H, 4 * SHARD), fp16)
        for i in range(HEAD):
            x_my_r[:, i * SHARD:(i + 1) * SHARD] = \
                x_my[32 * i:32 * (i + 1)].astype(fp16)
        in_maps.append(dict(common, x_shuf=x_shuf_cf, x_my=x_my,
                            x_my_r=x_my_r, A2=A2, lhsT_seed=lhsT_seed))
    vs = float(np.asarray(inputs["vss_skip"]).ravel()[0])
    cvm = float(np.asarray(inputs["cvm_skip"]).ravel()[0])
    return in_maps, vs, cvm


def kernel(**inputs) -> np.ndarray:
    from concourse.bass_utils import run_bass_kernel_spmd

    in_maps, vs, cvm = _host_prep(inputs)
    key = (vs, cvm)
    if key not in _cache:
        _cache[key] = _build(vs, cvm)
    nc = _cache[key]
    res = run_bass_kernel_spmd(nc, in_maps, core_ids=list(range(NCORES)))
    out = np.zeros((B, H, W, C_IN), np.float32)
    for b in range(B):
        full = np.zeros((C_IN, L), np.float32)
        for r in range(4):
            full[:, r * SHARD:(r + 1) * SHARD] = \
                res.results[4 * b + r]["out_cf"]
        out[b] = full.T.reshape(H, W, C_IN)
    return out
